# revision 9
# baseline (speedup 1.0000x reference)
"""NerfExperts MoE kernel for Trainium2, expert-parallel over 8 NeuronCores.

Each of the 1024 points is routed to one of 100 experts (~2.3MB fp32 of
weights each -> memory bound).  Experts are sharded across the 8 cores
(~13 slots/core); tokens are dispatched host-side; each expert's weights
stream from HBM exactly once in mixed precision:

  * w1-w7(mid), wi, wc0(inter) -> fp8 e3m4, scaled by a per-layer power
    of two (max |W| ~ 7, e3m4's normal band); the 2^-k descale rides the
    existing PSUM->SBUF bias move (scalar_tensor_tensor MUL+ADD).  The
    fp16 skip/rays slabs that accumulate into the same PSUM groups are
    pre-scaled by the same 2^k (exact in fp16).
  * w0, w5skip, wc0rays, wa, wc1 -> fp16.  biases fp32.  Activations
    stay bf16 (mixed-dtype matmul is legal; only fp32 must match).

This cuts weight DMA from ~16.8MB to ~8.0MB per core.  All e3m4 data
lives in ONE DRAM tensor streamed as three ~2.5MB column slices: the
HWDGE descriptor-generation rate (~17 GB/s per KB of per-partition
descriptor size) caps small transfers, so slices are kept fat.  The two
courier queues -- sync (HWDGE) + gpsimd (SWDGE) -- never run
compute-dependent work, so chunk kicks are never delayed behind
activations (the ACT engine only gets the tiny xyz DMAs).

Embedding slabs are exact-height (39 rows pts / 27 rows dirs): the
harmonic sin AND cos are produced by ONE fused chain -- the frequency
matmul gets a constant pi/2 "phase row" (ones row in the points tensor x
a [0 | pi/2] row in the freq matrix), so cos rows land at 18:36 without
any partition-base-shifting copies.

Embedding tile rows: pts: sin 0:18, cos 18:36, xyz 36:39;
dirs: sin 0:12, cos 12:24, xyz 24:27.
"""

import numpy as np
import ml_dtypes

import concourse.bass as bass
import concourse.bacc as bacc
import concourse.mybir as mybir
import concourse.tile as tile
from concourse.bass_utils import run_bass_kernel_spmd

PI = float(np.pi)
N_CORES = 8
E = 100
NX, ND = 6, 4
CAP_MAX = 128  # max tokens per expert slot (keeps matmul N and PSUM in range)
TARGET = 10.0  # e3m4 scale target for max|W| (normal band, <15.5)

# fp8 e3m4 mid stages (stage number = chunk name); 8 is wi, 10 is wc0
E3_STAGES = (1, 2, 3, 4, 5, 6, 7, 8)
# e3m4 mega-tensor column blocks per slot (stage-major): w1..w7 mid, wi, wc0i
E3_COLS = 8 * 512 + 256          # per-slot columns in the e3m4 mega tensor

# fp32 bias tensor [128, 21*nslot], layer-major columns:
#   mlp stage lidx in 0..8 (layers 0-7, then wi): col = lidx*2*nslot + s*2 + j
#   ba: 18*nslot + s ; bc0: 19*nslot + s ; bc1: 20*nslot + s
NB = 21

P_COLS = 640   # per slot: w0 256 | w5skip 256 | rays 128  (39 rows, fp16)
TL_COLS = 5    # per slot: wa 2 | wc1 3                    (128 rows, fp16)


def _e3_off(st, nslot):
    # column offset of stage block in the e3m4 mega tensor
    # st in 1..7 -> mid layers; 8 -> wi; 10 -> wc0 inter
    if st == 10:
        return 8 * 512 * nslot
    return (st - 1) * 512 * nslot


def _pack_expert(chunks, bt, s, nslot, inputs, e, scales):
    """Fill slot s of the per-chunk host arrays (fp32; cast later)."""
    n2 = 2 * nslot

    def set_b2(lidx, b):
        bt[:, lidx * n2 + s * 2] = b[0:128]
        bt[:, lidx * n2 + s * 2 + 1] = b[128:256]

    # --- P chunk (39 rows): w0 | w5 skip part | wc0 rays part.  skip and
    # rays are pre-scaled by their stage's 2^k so the shared PSUM
    # accumulation with the e3m4 slabs stays uniform. ---
    p = chunks["P"]
    o = s * P_COLS
    p[0:39, o: o + 256] = inputs["w0"][e]                 # [39, 256]
    p[0:39, o + 256: o + 512] = inputs["w5"][e][256:295] * 2.0 ** scales[5]
    p[0:27, o + 512: o + 640] = inputs["wc0"][e][256:283] * 2.0 ** scales[10]
    set_b2(0, inputs["b0"][e])
    # --- e3m4 mega tensor: 512-col blocks (k-split halves of [256,256]) ---
    w8 = chunks["E3"]
    for st, name in ((1, "w1"), (2, "w2"), (3, "w3"), (4, "w4"),
                     (5, "w5"), (6, "w6"), (7, "w7"), (8, "wi")):
        w = inputs[name][e][0:256]
        sc = 2.0 ** scales[st]
        o = _e3_off(st, nslot) + s * 512
        for k in (0, 1):
            w8[:, o + k * 256: o + (k + 1) * 256] = \
                w[128 * k: 128 * (k + 1)] * sc
        set_b2(st, inputs["b5" if name == "w5" else
                          ("bi" if name == "wi" else f"b{st}")][e])
    wc0 = inputs["wc0"][e]
    o = _e3_off(10, nslot) + s * 256
    sc = 2.0 ** scales[10]
    w8[:, o: o + 128] = wc0[0:128] * sc
    w8[:, o + 128: o + 256] = wc0[128:256] * sc
    bt[:, 19 * nslot + s] = inputs["bc0"][e]
    # --- tiny fp16 tail: wa | wc1 ---
    t = chunks["T"]
    o = s * TL_COLS
    wa = inputs["wa"][e][:, 0]
    t[:, o] = wa[0:128]
    t[:, o + 1] = wa[128:256]
    bt[0, 18 * nslot + s] = inputs["ba"][e][0]
    t[:, o + 2: o + 5] = inputs["wc1"][e]
    bt[0:3, 20 * nslot + s] = inputs["bc1"][e]


def _make_waves(nslot, C):
    gmax = max(1, min(512 // (2 * C), 6))
    nw = int(np.ceil(nslot / gmax))
    base = nslot // nw
    rem = nslot - base * nw
    sizes = [base + (1 if i < rem else 0) for i in range(nw)]
    waves, s0 = [], 0
    for g in sizes:
        waves.append((s0, s0 + g))
        s0 += g
    return waves


# ---------------------------------------------------------------------------
# Device program
# ---------------------------------------------------------------------------

def _build_program(C, nslot, scales):
    """SPMD Bass program: nslot expert slots of C tokens each."""
    nall = nslot * C
    waves = _make_waves(nslot, C)
    nw = len(waves)
    f32 = mybir.dt.float32
    f16 = mybir.dt.float16
    bf16 = mybir.dt.bfloat16
    f8e3 = mybir.dt.float8e3
    Sin = mybir.ActivationFunctionType.Sin
    Sigmoid = mybir.ActivationFunctionType.Sigmoid
    Relu = mybir.ActivationFunctionType.Relu
    ADD = mybir.AluOpType.add
    SUB = mybir.AluOpType.subtract
    MUL = mybir.AluOpType.mult
    MAX = mybir.AluOpType.max
    MIN = mybir.AluOpType.min
    # range-reduction constants (Cody-Waite, fp32 magic rounding)
    INV2PI = float(np.float32(1.0 / (2 * PI)))
    MAGIC = 12582912.0            # 1.5 * 2**23: forces round-to-int in fp32
    C1 = 6.28125                  # 2*pi high part, exact in fp32
    C2 = float(np.float32(2 * PI - 6.28125))
    CLAMP = 3.1415925             # just under pi (ACT Sin domain is [-pi, pi])

    nc = bacc.Bacc("TRN2", target_bir_lowering=False, debug=False)
    pd_d = nc.dram_tensor("pd", (4, 2 * nall + 60), f32, kind="ExternalInput")
    xyz_d = nc.dram_tensor("xyzb", (6, nall), bf16, kind="ExternalInput")
    bt_d = nc.dram_tensor("bt", (128, NB * nslot), f32, kind="ExternalInput")
    wp_d = nc.dram_tensor("wP", (39, nslot * P_COLS), f16, kind="ExternalInput")
    we3_d = nc.dram_tensor("wE3", (128, nslot * E3_COLS), f8e3,
                           kind="ExternalInput")
    wt_d = nc.dram_tensor("wT", (128, nslot * TL_COLS), f16,
                          kind="ExternalInput")
    al_d = nc.dram_tensor("alpha_out", (1, nall), f32, kind="ExternalOutput")
    co_d = nc.dram_tensor("color_out", (3, nall), f32, kind="ExternalOutput")

    with tile.TileContext(nc) as tc:
        with (
            tc.tile_pool(name="cp", bufs=1) as cp,
            tc.tile_pool(name="xp", bufs=2 * nw + 2) as xp,
            tc.tile_pool(name="psA", bufs=6, space=bass.MemorySpace.PSUM) as psA,
            tc.tile_pool(name="psB", bufs=2, space=bass.MemorySpace.PSUM) as psB,
        ):
            embP = cp.tile([39, nall], bf16)  # sin 0:18, cos 18:36, xyz 36:39
            embD = cp.tile([27, nall], bf16)  # sin 0:12, cos 12:24, xyz 24:27
            pd_sb = cp.tile([4, 2 * nall + 60], f32)
            bt_sb = cp.tile([128, NB * nslot], f32)
            wp_sb = cp.tile([39, nslot * P_COLS], f16, name="wP", tag="wP")
            we3_sb = cp.tile([128, nslot * E3_COLS], f8e3, name="wE3",
                             tag="wE3")
            wt_sb = cp.tile([128, nslot * TL_COLS], f16, name="wT", tag="wT")

            # ---- DMA kicks.  sync + gpsimd are pure couriers (no compute
            # deps ever block their queues); ACT gets only the tiny xyz
            # rows.  The e3m4 mega tensor streams as three fat column
            # slices (>=2MB each keeps the per-descriptor DGE rate off the
            # critical path); slice boundaries follow consumption order. ----
            nc.sync.dma_start(pd_sb[:], pd_d.ap()[:])
            nc.scalar.dma_start(embP[36:39, :], xyz_d.ap()[0:3, :])
            nc.scalar.dma_start(embD[24:27, :], xyz_d.ap()[3:6, :])
            nc.gpsimd.dma_start(bt_sb[:], bt_d.ap()[:])
            nc.gpsimd.dma_start(wt_sb[:], wt_d.ap()[:])
            nc.gpsimd.dma_start(wp_sb[:], wp_d.ap()[:])
            c1 = _e3_off(4, nslot)            # slice 1: w1,w2,w3
            c2 = _e3_off(7, nslot)            # slice 2: w4,w5,w6
            c3 = nslot * E3_COLS              # slice 3: w7,wi,wc0i
            nc.sync.dma_start(we3_sb[:, 0:c1], we3_d.ap()[:, 0:c1])
            nc.gpsimd.dma_start(we3_sb[:, c1:c2], we3_d.ap()[:, c1:c2])
            nc.sync.dma_start(we3_sb[:, c2:c3], we3_d.ap()[:, c2:c3])

            alpha_sb = cp.tile([1, nall], f32)
            color_sb = cp.tile([3, nall], f32)

            # ---- harmonic embedding: ONE fused sin+cos chain per source.
            # ep rows 0:half are freq*x, rows half:2*half are freq*x + pi/2
            # (phase row trick), so Sin() yields sin|cos in one shot. ----
            def reduce_sin(tsrc, rows, ncol):
                t1 = xp.tile([rows, ncol], f32, tag="vred")
                nc.vector.tensor_scalar(t1[:], tsrc, INV2PI, MAGIC, MUL, ADD)
                r = xp.tile([rows, ncol], f32, tag="vred")
                nc.vector.tensor_scalar(r[:], t1[:], MAGIC, None, SUB)
                rd = xp.tile([rows, ncol], f32, tag="vred")
                nc.vector.scalar_tensor_tensor(rd[:], r[:], -C1, tsrc, MUL, ADD)
                rd2 = xp.tile([rows, ncol], f32, tag="vred")
                nc.vector.scalar_tensor_tensor(rd2[:], r[:], -C2, rd[:], MUL, ADD)
                v = xp.tile([rows, ncol], f32, tag="vred")
                nc.vector.tensor_scalar(v[:], rd2[:], CLAMP, -CLAMP, MIN, MAX)
                return v

            for lo in range(0, nall, 512):
                hi = min(nall, lo + 512)
                w_ = hi - lo
                for (rows, fcol, src_lo, dst) in (
                    (36, 2 * nall, 0, embP),
                    (24, 2 * nall + 36, nall, embD),
                ):
                    ep = psA.tile([rows, w_], f32, tag="mlp")
                    nc.tensor.matmul(ep[:], pd_sb[0:4, fcol: fcol + rows],
                                     pd_sb[0:4, src_lo + lo: src_lo + hi],
                                     start=True, stop=True)
                    vs = reduce_sin(ep[:], rows, w_)
                    nc.scalar.activation(dst[0:rows, lo:hi], vs[:], Sin)

            # ---- wave-lockstep MLP ----
            def bias2_bcast(lidx, s0, s1):
                # 3D: [p, (g j), C] -- bias cols are (slot, j)-ordered, which
                # matches the slot-major-j-inner psum/x layout
                g = s1 - s0
                ap = bt_sb[:, lidx * 2 * nslot + s0 * 2: lidx * 2 * nslot + s1 * 2]
                return ap.broadcast_to([128, 2 * g, C])

            def bias1_bcast(which, s0, s1, p=128):
                g = s1 - s0
                ap = bt_sb[0:p, which * nslot + s0: which * nslot + s1]
                return ap.broadcast_to([p, g, C])

            xs = [None] * nw
            its = [None] * nw
            cts = [None] * nw

            def slab(st, s, lo, hi):
                o = _e3_off(st, nslot) + s * (256 if st == 10 else 512)
                return we3_sb[0:128, o + lo: o + hi]

            def xsl(t, i, j):
                return t[:, (2 * i + j) * C:(2 * i + j + 1) * C]

            def mm_mid(st, ps, xin, s0, s1):
                for i in range(s1 - s0):
                    s = s0 + i
                    for j in (0, 1):
                        pj = xsl(ps, i, j)
                        nc.tensor.matmul(pj, slab(st, s, j * 128, j * 128 + 128),
                                         xsl(xin, i, 0),
                                         start=True, stop=False)
                        nc.tensor.matmul(pj, slab(st, s, 256 + j * 128, 256 + j * 128 + 128),
                                         xsl(xin, i, 1),
                                         start=False, stop=True)

            def move2(ps, lidx, s0, s1, relu=True):
                g = s1 - s0
                xn = xp.tile([128, 2 * g * C], bf16, tag="x")
                psv = ps[:].rearrange("p (a c) -> p a c", a=2 * g)
                xnv = xn[:].rearrange("p (a c) -> p a c", a=2 * g)
                if lidx in E3_STAGES:
                    dsc = float(2.0 ** (-scales[lidx]))
                    nc.vector.scalar_tensor_tensor(
                        xnv, psv, dsc, bias2_bcast(lidx, s0, s1), MUL, ADD)
                else:
                    nc.vector.tensor_tensor(
                        xnv, psv, bias2_bcast(lidx, s0, s1), ADD)
                if relu:
                    nc.scalar.activation(xn[:], xn[:], Relu)
                return xn

            def emit_stage(wi_, stage):
                s0, s1 = waves[wi_]
                g = s1 - s0
                if stage == 0:  # w0 (fp16, 39-row slabs)
                    ps = psA.tile([128, 2 * g * C], f32, tag="mlp")
                    for i in range(g):
                        s = s0 + i
                        sl = slice(s * C, (s + 1) * C)
                        o = s * P_COLS
                        for j in (0, 1):
                            nc.tensor.matmul(
                                xsl(ps, i, j),
                                wp_sb[0:39, o + j * 128: o + j * 128 + 128],
                                embP[0:39, sl], start=True, stop=True)
                    xs[wi_] = move2(ps, 0, s0, s1)
                elif stage in (1, 2, 3, 4, 6, 7):
                    ps = psA.tile([128, 2 * g * C], f32, tag="mlp")
                    mm_mid(stage, ps, xs[wi_], s0, s1)
                    xs[wi_] = move2(ps, stage, s0, s1)
                elif stage == 5:  # fp16 mid + fp16 skip slab
                    ps = psA.tile([128, 2 * g * C], f32, tag="mlp")
                    xin = xs[wi_]
                    for i in range(g):
                        s = s0 + i
                        sl = slice(s * C, (s + 1) * C)
                        o = s * P_COLS
                        for j in (0, 1):
                            pj = xsl(ps, i, j)
                            nc.tensor.matmul(pj, slab(5, s, j * 128, j * 128 + 128),
                                             xsl(xin, i, 0),
                                             start=True, stop=False)
                            nc.tensor.matmul(pj, slab(5, s, 256 + j * 128, 256 + j * 128 + 128),
                                             xsl(xin, i, 1),
                                             start=False, stop=False)
                            # (skip slab is pre-scaled by 2^k5 on the host)
                            nc.tensor.matmul(
                                pj,
                                wp_sb[0:39, o + 256 + j * 128: o + 384 + j * 128],
                                embP[0:39, sl], start=False, stop=True)
                    xs[wi_] = move2(ps, 5, s0, s1)
                elif stage == 8:  # wi -> inter (e3m4; bias, no relu)
                    ps = psA.tile([128, 2 * g * C], f32, tag="mlp")
                    mm_mid(8, ps, xs[wi_], s0, s1)
                    its[wi_] = move2(ps, 8, s0, s1, relu=False)
                elif stage == 9:  # wa -> alpha (fp16, in tail chunk)
                    pa = psB.tile([3, g * C], f32, tag="head")
                    xin = xs[wi_]
                    for i in range(g):
                        s = s0 + i
                        o = s * TL_COLS
                        nc.tensor.matmul(pa[0:1, i * C:(i + 1) * C],
                                         wt_sb[:, o: o + 1],
                                         xsl(xin, i, 0),
                                         start=True, stop=False)
                        nc.tensor.matmul(pa[0:1, i * C:(i + 1) * C],
                                         wt_sb[:, o + 1: o + 2],
                                         xsl(xin, i, 1),
                                         start=False, stop=True)
                    av = alpha_sb[0:1, s0 * C: s1 * C].rearrange(
                        "p (g c) -> p g c", g=g)
                    pav = pa[0:1, :].rearrange("p (g c) -> p g c", g=g)
                    nc.vector.tensor_tensor(av, pav, bias1_bcast(18, s0, s1, p=1), ADD)
                elif stage == 10:  # wc0 -> c (fp16 + rays slab, relu)
                    pc = psA.tile([128, g * C], f32, tag="mlp")
                    it = its[wi_]
                    for i in range(g):
                        s = s0 + i
                        sl = slice(s * C, (s + 1) * C)
                        o = s * TL_COLS
                        op = s * P_COLS
                        pj = pc[:, i * C:(i + 1) * C]
                        nc.tensor.matmul(pj, slab(10, s, 0, 128),
                                         xsl(it, i, 0),
                                         start=True, stop=False)
                        nc.tensor.matmul(pj, slab(10, s, 128, 256),
                                         xsl(it, i, 1),
                                         start=False, stop=False)
                        nc.tensor.matmul(pj, wp_sb[0:27, op + 512: op + 640],
                                         embD[0:27, sl],
                                         start=False, stop=True)
                    ct = xp.tile([128, g * C], bf16, tag="ct")
                    pcv = pc[:].rearrange("p (g c) -> p g c", g=g)
                    ctv = ct[:].rearrange("p (g c) -> p g c", g=g)
                    nc.vector.scalar_tensor_tensor(
                        ctv, pcv, float(2.0 ** (-scales[10])),
                        bias1_bcast(19, s0, s1), MUL, ADD)
                    nc.scalar.activation(ct[:], ct[:], Relu)
                    cts[wi_] = ct
                elif stage == 11:  # wc1 -> sigmoid color (fp16, tail chunk)
                    pcol = psB.tile([3, g * C], f32, tag="head")
                    ct = cts[wi_]
                    for i in range(g):
                        s = s0 + i
                        o = s * TL_COLS
                        nc.tensor.matmul(pcol[:, i * C:(i + 1) * C],
                                         wt_sb[:, o + 2: o + 5],
                                         ct[:, i * C:(i + 1) * C],
                                         start=True, stop=True)
                    ctmp = xp.tile([3, g * C], f32, tag="ctmp")
                    pv = pcol[:].rearrange("p (g c) -> p g c", g=g)
                    cv = ctmp[:].rearrange("p (g c) -> p g c", g=g)
                    nc.vector.tensor_tensor(cv, pv, bias1_bcast(20, s0, s1, p=3), ADD)
                    nc.scalar.activation(color_sb[0:3, s0 * C: s1 * C], ctmp[:],
                                         Sigmoid)

            for stage in range(12):
                for wi_ in range(nw):
                    emit_stage(wi_, stage)

            nc.sync.dma_start(al_d.ap()[:], alpha_sb[:])
            nc.sync.dma_start(co_d.ap()[:], color_sb[:])

    nc.compile()
    return nc


_prog_cache = {}
_last_results = None


def _get_program(C, nslot, scales):
    key = (C, nslot, tuple(sorted(scales.items())))
    if key not in _prog_cache:
        _prog_cache[key] = _build_program(C, nslot, scales)
    return _prog_cache[key]


# ---------------------------------------------------------------------------
# Host wrapper
# ---------------------------------------------------------------------------

def kernel(**inputs):
    global _last_results
    inputs = {k: np.asarray(v) for k, v in inputs.items()}
    idx = inputs["index"].astype(np.int64)
    B = idx.shape[0]
    points = inputs["points"].astype(np.float32)
    dirs = inputs["directions"].astype(np.float32)

    # --- routing: split each expert's tokens into <=CAP_MAX chunks ("virtual
    # experts"), distribute round-robin (sorted by size) over 8 cores ---
    tok = [np.nonzero(idx == e)[0] for e in range(E)]
    virt = []  # (expert, token_ids)
    for e in range(E):
        t = tok[e]
        if len(t) == 0:
            continue
        for lo in range(0, len(t), CAP_MAX):
            virt.append((e, t[lo: lo + CAP_MAX]))
    if not virt:
        virt = [(0, np.zeros((0,), np.int64))]
    virt.sort(key=lambda v: -len(v[1]))
    nslot = max(1, int(np.ceil(len(virt) / N_CORES)))
    C = max(4, int(np.ceil(max(len(v[1]) for v in virt) / 4) * 4))
    nall = nslot * C

    core_slots = [[] for _ in range(N_CORES)]
    for i, v in enumerate(virt):
        core_slots[i % N_CORES].append(v)

    # per-layer global pow2 scales for the e3m4 stages (w5/wc0 scale over
    # the e3m4 rows only; their fp16 skip/rays slabs reuse the same k)
    scales = {}
    for st, name in ((1, "w1"), (2, "w2"), (3, "w3"), (4, "w4"),
                     (6, "w6"), (7, "w7"), (8, "wi")):
        gmax = float(np.abs(inputs[name]).max())
        scales[st] = float(np.floor(np.log2(TARGET / gmax)))
    scales[5] = float(np.floor(np.log2(
        TARGET / float(np.abs(inputs["w5"][:, 0:256]).max()))))
    scales[10] = float(np.floor(np.log2(
        TARGET / float(np.abs(inputs["wc0"][:, 0:256]).max()))))

    nc = _get_program(C, nslot, scales)

    # frequency expansion blocks with the pi/2 phase row (row 3):
    # pts: cols 0:18 sin, 18:36 cos; dirs: 0:12 sin, 12:24 cos
    fx2 = np.zeros((4, 36), np.float32)
    for c in range(3):
        for k in range(NX):
            fx2[c, c * NX + k] = float(2 ** k)
            fx2[c, 18 + c * NX + k] = float(2 ** k)
    fx2[3, 18:36] = PI / 2
    fd2 = np.zeros((4, 24), np.float32)
    for c in range(3):
        for k in range(ND):
            fd2[c, c * ND + k] = float(2 ** k)
            fd2[c, 12 + c * ND + k] = float(2 ** k)
    fd2[3, 12:24] = PI / 2

    e3 = ml_dtypes.float8_e3m4
    in_maps = []
    for cid in range(N_CORES):
        chunks = {"P": np.zeros((39, nslot * P_COLS), np.float32),
                  "T": np.zeros((128, nslot * TL_COLS), np.float32),
                  "E3": np.zeros((128, nslot * E3_COLS), np.float32)}
        bt = np.zeros((128, NB * nslot), np.float32)
        pd = np.zeros((4, 2 * nall + 60), np.float32)
        pd[3, 0: 2 * nall] = 1.0
        pd[:, 2 * nall: 2 * nall + 36] = fx2
        pd[:, 2 * nall + 36: 2 * nall + 60] = fd2
        xyzb = np.zeros((6, nall), np.float32)
        for s, (e, t) in enumerate(core_slots[cid]):
            _pack_expert(chunks, bt, s, nslot, inputs, e, scales)
            n = len(t)
            if n:
                pd[0:3, s * C: s * C + n] = points[t].T
                pd[0:3, nall + s * C: nall + s * C + n] = dirs[t].T
                xyzb[0:3, s * C: s * C + n] = points[t].T
                xyzb[3:6, s * C: s * C + n] = dirs[t].T
        im = {"pd": pd, "bt": bt,
              "xyzb": xyzb.astype(ml_dtypes.bfloat16),
              "wP": chunks["P"].astype(np.float16),
              "wT": chunks["T"].astype(np.float16),
              "wE3": chunks["E3"].astype(e3)}
        in_maps.append(im)

    res = run_bass_kernel_spmd(nc, in_maps, core_ids=list(range(N_CORES)))
    _last_results = res

    out = np.zeros((B, 4), np.float32)
    for cid in range(N_CORES):
        al = res.results[cid]["alpha_out"]
        co = res.results[cid]["color_out"]
        for s, (e, t) in enumerate(core_slots[cid]):
            n = len(t)
            if n:
                out[t, 0] = al[0, s * C: s * C + n]
                out[t, 1:4] = co[:, s * C: s * C + n].T
    return out


# revision 10
# speedup vs baseline: 1.1103x; 1.1103x over previous
"""NerfExperts MoE kernel for Trainium2, expert-parallel over 8 NeuronCores.

Each of the 1024 points is routed to one of 100 experts (~2.3MB fp32 of
weights each -> memory bound).  Experts are sharded across the 8 cores
(~13 slots/core); tokens are dispatched host-side; each expert's weights
stream from HBM exactly once in mixed precision:

  * w1-w7(mid), wi, wc0(inter) -> fp8 e3m4, scaled by a per-layer power
    of two (max |W| ~ 7, e3m4's normal band); the 2^-k descale rides the
    existing PSUM->SBUF bias move (scalar_tensor_tensor MUL+ADD).  The
    fp16 skip/rays slabs that accumulate into the same PSUM groups are
    pre-scaled by the same 2^k (exact in fp16).
  * w0, w5skip, wc0rays, wa, wc1 -> fp16.  biases fp32.  Activations
    stay bf16 (mixed-dtype matmul is legal; only fp32 must match).

This cuts weight DMA from ~16.8MB to ~8.0MB per core.  All e3m4 data
lives in ONE DRAM tensor streamed as ~1.3-2.6MB column slices over the
two HWDGE rings (sync + scalar).  SWDGE (gpsimd) is deliberately NOT
used: its SBUF-resident descriptor rings throttle every SDMA engine's
per-packet rate (~17.5 vs ~22.4 GB/s/engine measured), costing ~90GB/s
of aggregate HBM bandwidth.  All DMA kicks are emitted before any
compute so the ACT ring's kicks precede its activation work.

Embedding slabs are exact-height (39 rows pts / 27 rows dirs): the
harmonic sin AND cos are produced by ONE fused chain -- the frequency
matmul gets a constant pi/2 "phase row" (ones row in the points tensor x
a [0 | pi/2] row in the freq matrix), so cos rows land at 18:36 without
any partition-base-shifting copies.

Embedding tile rows: pts: sin 0:18, cos 18:36, xyz 36:39;
dirs: sin 0:12, cos 12:24, xyz 24:27.
"""

import numpy as np
import ml_dtypes

import concourse.bass as bass
import concourse.bacc as bacc
import concourse.mybir as mybir
import concourse.tile as tile
from concourse.bass_utils import run_bass_kernel_spmd

PI = float(np.pi)
N_CORES = 8
E = 100
NX, ND = 6, 4
CAP_MAX = 128  # max tokens per expert slot (keeps matmul N and PSUM in range)
TARGET = 10.0  # e3m4 scale target for max|W| (normal band, <15.5)

# fp8 e3m4 mid stages (stage number = chunk name); 8 is wi, 10 is wc0
E3_STAGES = (1, 2, 3, 4, 5, 6, 7, 8)
# e3m4 mega-tensor column blocks per slot (stage-major): w1..w7 mid, wi, wc0i
E3_COLS = 8 * 512 + 256          # per-slot columns in the e3m4 mega tensor

# fp32 bias tensor [128, 21*nslot], layer-major columns:
#   mlp stage lidx in 0..8 (layers 0-7, then wi): col = lidx*2*nslot + s*2 + j
#   ba: 18*nslot + s ; bc0: 19*nslot + s ; bc1: 20*nslot + s
NB = 21

P_COLS = 640   # per slot: w0 256 | w5skip 256 | rays 128  (39 rows, fp16)
TL_COLS = 5    # per slot: wa 2 | wc1 3                    (128 rows, fp16)


def _e3_off(st, nslot):
    # column offset of stage block in the e3m4 mega tensor
    # st in 1..7 -> mid layers; 8 -> wi; 10 -> wc0 inter
    if st == 10:
        return 8 * 512 * nslot
    return (st - 1) * 512 * nslot


def _pack_expert(chunks, bt, s, nslot, inputs, e, scales):
    """Fill slot s of the per-chunk host arrays (fp32; cast later)."""
    n2 = 2 * nslot

    def set_b2(lidx, b):
        bt[:, lidx * n2 + s * 2] = b[0:128]
        bt[:, lidx * n2 + s * 2 + 1] = b[128:256]

    # --- P chunk (39 rows): w0 | w5 skip part | wc0 rays part.  skip and
    # rays are pre-scaled by their stage's 2^k so the shared PSUM
    # accumulation with the e3m4 slabs stays uniform. ---
    p = chunks["P"]
    o = s * P_COLS
    p[0:39, o: o + 256] = inputs["w0"][e]                 # [39, 256]
    p[0:39, o + 256: o + 512] = inputs["w5"][e][256:295] * 2.0 ** scales[5]
    p[0:27, o + 512: o + 640] = inputs["wc0"][e][256:283] * 2.0 ** scales[10]
    set_b2(0, inputs["b0"][e])
    # --- e3m4 mega tensor: 512-col blocks (k-split halves of [256,256]) ---
    w8 = chunks["E3"]
    for st, name in ((1, "w1"), (2, "w2"), (3, "w3"), (4, "w4"),
                     (5, "w5"), (6, "w6"), (7, "w7"), (8, "wi")):
        w = inputs[name][e][0:256]
        sc = 2.0 ** scales[st]
        o = _e3_off(st, nslot) + s * 512
        for k in (0, 1):
            w8[:, o + k * 256: o + (k + 1) * 256] = \
                w[128 * k: 128 * (k + 1)] * sc
        set_b2(st, inputs["b5" if name == "w5" else
                          ("bi" if name == "wi" else f"b{st}")][e])
    wc0 = inputs["wc0"][e]
    o = _e3_off(10, nslot) + s * 256
    sc = 2.0 ** scales[10]
    w8[:, o: o + 128] = wc0[0:128] * sc
    w8[:, o + 128: o + 256] = wc0[128:256] * sc
    bt[:, 19 * nslot + s] = inputs["bc0"][e]
    # --- tiny fp16 tail: wa | wc1 ---
    t = chunks["T"]
    o = s * TL_COLS
    wa = inputs["wa"][e][:, 0]
    t[:, o] = wa[0:128]
    t[:, o + 1] = wa[128:256]
    bt[0, 18 * nslot + s] = inputs["ba"][e][0]
    t[:, o + 2: o + 5] = inputs["wc1"][e]
    bt[0:3, 20 * nslot + s] = inputs["bc1"][e]


def _make_waves(nslot, C):
    gmax = max(1, min(512 // (2 * C), 6))
    nw = int(np.ceil(nslot / gmax))
    base = nslot // nw
    rem = nslot - base * nw
    sizes = [base + (1 if i < rem else 0) for i in range(nw)]
    waves, s0 = [], 0
    for g in sizes:
        waves.append((s0, s0 + g))
        s0 += g
    return waves


# ---------------------------------------------------------------------------
# Device program
# ---------------------------------------------------------------------------

def _build_program(C, nslot, scales):
    """SPMD Bass program: nslot expert slots of C tokens each."""
    nall = nslot * C
    waves = _make_waves(nslot, C)
    nw = len(waves)
    f32 = mybir.dt.float32
    f16 = mybir.dt.float16
    bf16 = mybir.dt.bfloat16
    f8e3 = mybir.dt.float8e3
    Sin = mybir.ActivationFunctionType.Sin
    Sigmoid = mybir.ActivationFunctionType.Sigmoid
    Relu = mybir.ActivationFunctionType.Relu
    ADD = mybir.AluOpType.add
    SUB = mybir.AluOpType.subtract
    MUL = mybir.AluOpType.mult
    MAX = mybir.AluOpType.max
    MIN = mybir.AluOpType.min
    # range-reduction constants (Cody-Waite, fp32 magic rounding)
    INV2PI = float(np.float32(1.0 / (2 * PI)))
    MAGIC = 12582912.0            # 1.5 * 2**23: forces round-to-int in fp32
    C1 = 6.28125                  # 2*pi high part, exact in fp32
    C2 = float(np.float32(2 * PI - 6.28125))
    CLAMP = 3.1415925             # just under pi (ACT Sin domain is [-pi, pi])

    nc = bacc.Bacc("TRN2", target_bir_lowering=False, debug=False)
    pd_d = nc.dram_tensor("pd", (4, 2 * nall + 60), f32, kind="ExternalInput")
    xyz_d = nc.dram_tensor("xyzb", (6, nall), bf16, kind="ExternalInput")
    bt_d = nc.dram_tensor("bt", (128, NB * nslot), f32, kind="ExternalInput")
    wp_d = nc.dram_tensor("wP", (39, nslot * P_COLS), f16, kind="ExternalInput")
    we3_d = nc.dram_tensor("wE3", (128, nslot * E3_COLS), f8e3,
                           kind="ExternalInput")
    wt_d = nc.dram_tensor("wT", (128, nslot * TL_COLS), f16,
                          kind="ExternalInput")
    al_d = nc.dram_tensor("alpha_out", (1, nall), f32, kind="ExternalOutput")
    co_d = nc.dram_tensor("color_out", (3, nall), f32, kind="ExternalOutput")

    with tile.TileContext(nc) as tc:
        with (
            tc.tile_pool(name="cp", bufs=1) as cp,
            tc.tile_pool(name="xp", bufs=2 * nw + 2) as xp,
            tc.tile_pool(name="psA", bufs=6, space=bass.MemorySpace.PSUM) as psA,
            tc.tile_pool(name="psB", bufs=2, space=bass.MemorySpace.PSUM) as psB,
        ):
            embP = cp.tile([39, nall], bf16)  # sin 0:18, cos 18:36, xyz 36:39
            embD = cp.tile([27, nall], bf16)  # sin 0:12, cos 12:24, xyz 24:27
            pd_sb = cp.tile([4, 2 * nall + 60], f32)
            bt_sb = cp.tile([128, NB * nslot], f32)
            wp_sb = cp.tile([39, nslot * P_COLS], f16, name="wP", tag="wP")
            we3_sb = cp.tile([128, nslot * E3_COLS], f8e3, name="wE3",
                             tag="wE3")
            wt_sb = cp.tile([128, nslot * TL_COLS], f16, name="wT", tag="wT")

            # ---- DMA kicks.  sync + gpsimd are pure couriers (no compute
            # deps ever block their queues); ACT gets only the tiny xyz
            # rows.  The e3m4 mega tensor streams as three fat column
            # slices (>=2MB each keeps the per-descriptor DGE rate off the
            # critical path); slice boundaries follow consumption order. ----
            nc.sync.dma_start(pd_sb[:], pd_d.ap()[:])
            nc.scalar.dma_start(embP[36:39, :], xyz_d.ap()[0:3, :])
            nc.scalar.dma_start(embD[24:27, :], xyz_d.ap()[3:6, :])
            nc.scalar.dma_start(bt_sb[:], bt_d.ap()[:])
            nc.scalar.dma_start(wt_sb[:], wt_d.ap()[:])
            nc.scalar.dma_start(wp_sb[:], wp_d.ap()[:])
            c1 = _e3_off(4, nslot)            # slice 1: w1,w2,w3   (sync)
            c2 = _e3_off(6, nslot)            # slice 2: w4,w5     (scalar)
            c3 = _e3_off(8, nslot)            # slice 3: w6,w7      (sync)
            c4 = nslot * E3_COLS              # slice 4: wi,wc0i   (scalar)
            nc.sync.dma_start(we3_sb[:, 0:c1], we3_d.ap()[:, 0:c1])
            nc.scalar.dma_start(we3_sb[:, c1:c2], we3_d.ap()[:, c1:c2])
            nc.sync.dma_start(we3_sb[:, c2:c3], we3_d.ap()[:, c2:c3])
            nc.scalar.dma_start(we3_sb[:, c3:c4], we3_d.ap()[:, c3:c4])

            alpha_sb = cp.tile([1, nall], f32)
            color_sb = cp.tile([3, nall], f32)

            # ---- harmonic embedding: ONE fused sin+cos chain per source.
            # ep rows 0:half are freq*x, rows half:2*half are freq*x + pi/2
            # (phase row trick), so Sin() yields sin|cos in one shot. ----
            def reduce_sin(tsrc, rows, ncol):
                t1 = xp.tile([rows, ncol], f32, tag="vred")
                nc.vector.tensor_scalar(t1[:], tsrc, INV2PI, MAGIC, MUL, ADD)
                r = xp.tile([rows, ncol], f32, tag="vred")
                nc.vector.tensor_scalar(r[:], t1[:], MAGIC, None, SUB)
                rd = xp.tile([rows, ncol], f32, tag="vred")
                nc.vector.scalar_tensor_tensor(rd[:], r[:], -C1, tsrc, MUL, ADD)
                rd2 = xp.tile([rows, ncol], f32, tag="vred")
                nc.vector.scalar_tensor_tensor(rd2[:], r[:], -C2, rd[:], MUL, ADD)
                v = xp.tile([rows, ncol], f32, tag="vred")
                nc.vector.tensor_scalar(v[:], rd2[:], CLAMP, -CLAMP, MIN, MAX)
                return v

            for lo in range(0, nall, 512):
                hi = min(nall, lo + 512)
                w_ = hi - lo
                for (rows, fcol, src_lo, dst) in (
                    (36, 2 * nall, 0, embP),
                    (24, 2 * nall + 36, nall, embD),
                ):
                    ep = psA.tile([rows, w_], f32, tag="mlp")
                    nc.tensor.matmul(ep[:], pd_sb[0:4, fcol: fcol + rows],
                                     pd_sb[0:4, src_lo + lo: src_lo + hi],
                                     start=True, stop=True)
                    vs = reduce_sin(ep[:], rows, w_)
                    nc.scalar.activation(dst[0:rows, lo:hi], vs[:], Sin)

            # ---- wave-lockstep MLP ----
            def bias2_bcast(lidx, s0, s1):
                # 3D: [p, (g j), C] -- bias cols are (slot, j)-ordered, which
                # matches the slot-major-j-inner psum/x layout
                g = s1 - s0
                ap = bt_sb[:, lidx * 2 * nslot + s0 * 2: lidx * 2 * nslot + s1 * 2]
                return ap.broadcast_to([128, 2 * g, C])

            def bias1_bcast(which, s0, s1, p=128):
                g = s1 - s0
                ap = bt_sb[0:p, which * nslot + s0: which * nslot + s1]
                return ap.broadcast_to([p, g, C])

            xs = [None] * nw
            its = [None] * nw
            cts = [None] * nw

            def slab(st, s, lo, hi):
                o = _e3_off(st, nslot) + s * (256 if st == 10 else 512)
                return we3_sb[0:128, o + lo: o + hi]

            def xsl(t, i, j):
                return t[:, (2 * i + j) * C:(2 * i + j + 1) * C]

            def mm_mid(st, ps, xin, s0, s1):
                for i in range(s1 - s0):
                    s = s0 + i
                    for j in (0, 1):
                        pj = xsl(ps, i, j)
                        nc.tensor.matmul(pj, slab(st, s, j * 128, j * 128 + 128),
                                         xsl(xin, i, 0),
                                         start=True, stop=False)
                        nc.tensor.matmul(pj, slab(st, s, 256 + j * 128, 256 + j * 128 + 128),
                                         xsl(xin, i, 1),
                                         start=False, stop=True)

            def move2(ps, lidx, s0, s1, relu=True):
                g = s1 - s0
                xn = xp.tile([128, 2 * g * C], bf16, tag="x")
                psv = ps[:].rearrange("p (a c) -> p a c", a=2 * g)
                xnv = xn[:].rearrange("p (a c) -> p a c", a=2 * g)
                if lidx in E3_STAGES:
                    dsc = float(2.0 ** (-scales[lidx]))
                    nc.vector.scalar_tensor_tensor(
                        xnv, psv, dsc, bias2_bcast(lidx, s0, s1), MUL, ADD)
                else:
                    nc.vector.tensor_tensor(
                        xnv, psv, bias2_bcast(lidx, s0, s1), ADD)
                if relu:
                    nc.scalar.activation(xn[:], xn[:], Relu)
                return xn

            def emit_stage(wi_, stage):
                s0, s1 = waves[wi_]
                g = s1 - s0
                if stage == 0:  # w0 (fp16, 39-row slabs)
                    ps = psA.tile([128, 2 * g * C], f32, tag="mlp")
                    for i in range(g):
                        s = s0 + i
                        sl = slice(s * C, (s + 1) * C)
                        o = s * P_COLS
                        for j in (0, 1):
                            nc.tensor.matmul(
                                xsl(ps, i, j),
                                wp_sb[0:39, o + j * 128: o + j * 128 + 128],
                                embP[0:39, sl], start=True, stop=True)
                    xs[wi_] = move2(ps, 0, s0, s1)
                elif stage in (1, 2, 3, 4, 6, 7):
                    ps = psA.tile([128, 2 * g * C], f32, tag="mlp")
                    mm_mid(stage, ps, xs[wi_], s0, s1)
                    xs[wi_] = move2(ps, stage, s0, s1)
                elif stage == 5:  # fp16 mid + fp16 skip slab
                    ps = psA.tile([128, 2 * g * C], f32, tag="mlp")
                    xin = xs[wi_]
                    for i in range(g):
                        s = s0 + i
                        sl = slice(s * C, (s + 1) * C)
                        o = s * P_COLS
                        for j in (0, 1):
                            pj = xsl(ps, i, j)
                            nc.tensor.matmul(pj, slab(5, s, j * 128, j * 128 + 128),
                                             xsl(xin, i, 0),
                                             start=True, stop=False)
                            nc.tensor.matmul(pj, slab(5, s, 256 + j * 128, 256 + j * 128 + 128),
                                             xsl(xin, i, 1),
                                             start=False, stop=False)
                            # (skip slab is pre-scaled by 2^k5 on the host)
                            nc.tensor.matmul(
                                pj,
                                wp_sb[0:39, o + 256 + j * 128: o + 384 + j * 128],
                                embP[0:39, sl], start=False, stop=True)
                    xs[wi_] = move2(ps, 5, s0, s1)
                elif stage == 8:  # wi -> inter (e3m4; bias, no relu)
                    ps = psA.tile([128, 2 * g * C], f32, tag="mlp")
                    mm_mid(8, ps, xs[wi_], s0, s1)
                    its[wi_] = move2(ps, 8, s0, s1, relu=False)
                elif stage == 9:  # wa -> alpha (fp16, in tail chunk)
                    pa = psB.tile([3, g * C], f32, tag="head")
                    xin = xs[wi_]
                    for i in range(g):
                        s = s0 + i
                        o = s * TL_COLS
                        nc.tensor.matmul(pa[0:1, i * C:(i + 1) * C],
                                         wt_sb[:, o: o + 1],
                                         xsl(xin, i, 0),
                                         start=True, stop=False)
                        nc.tensor.matmul(pa[0:1, i * C:(i + 1) * C],
                                         wt_sb[:, o + 1: o + 2],
                                         xsl(xin, i, 1),
                                         start=False, stop=True)
                    av = alpha_sb[0:1, s0 * C: s1 * C].rearrange(
                        "p (g c) -> p g c", g=g)
                    pav = pa[0:1, :].rearrange("p (g c) -> p g c", g=g)
                    nc.vector.tensor_tensor(av, pav, bias1_bcast(18, s0, s1, p=1), ADD)
                elif stage == 10:  # wc0 -> c (fp16 + rays slab, relu)
                    pc = psA.tile([128, g * C], f32, tag="mlp")
                    it = its[wi_]
                    for i in range(g):
                        s = s0 + i
                        sl = slice(s * C, (s + 1) * C)
                        o = s * TL_COLS
                        op = s * P_COLS
                        pj = pc[:, i * C:(i + 1) * C]
                        nc.tensor.matmul(pj, slab(10, s, 0, 128),
                                         xsl(it, i, 0),
                                         start=True, stop=False)
                        nc.tensor.matmul(pj, slab(10, s, 128, 256),
                                         xsl(it, i, 1),
                                         start=False, stop=False)
                        nc.tensor.matmul(pj, wp_sb[0:27, op + 512: op + 640],
                                         embD[0:27, sl],
                                         start=False, stop=True)
                    ct = xp.tile([128, g * C], bf16, tag="ct")
                    pcv = pc[:].rearrange("p (g c) -> p g c", g=g)
                    ctv = ct[:].rearrange("p (g c) -> p g c", g=g)
                    nc.vector.scalar_tensor_tensor(
                        ctv, pcv, float(2.0 ** (-scales[10])),
                        bias1_bcast(19, s0, s1), MUL, ADD)
                    nc.scalar.activation(ct[:], ct[:], Relu)
                    cts[wi_] = ct
                elif stage == 11:  # wc1 -> sigmoid color (fp16, tail chunk)
                    pcol = psB.tile([3, g * C], f32, tag="head")
                    ct = cts[wi_]
                    for i in range(g):
                        s = s0 + i
                        o = s * TL_COLS
                        nc.tensor.matmul(pcol[:, i * C:(i + 1) * C],
                                         wt_sb[:, o + 2: o + 5],
                                         ct[:, i * C:(i + 1) * C],
                                         start=True, stop=True)
                    ctmp = xp.tile([3, g * C], f32, tag="ctmp")
                    pv = pcol[:].rearrange("p (g c) -> p g c", g=g)
                    cv = ctmp[:].rearrange("p (g c) -> p g c", g=g)
                    nc.vector.tensor_tensor(cv, pv, bias1_bcast(20, s0, s1, p=3), ADD)
                    nc.scalar.activation(color_sb[0:3, s0 * C: s1 * C], ctmp[:],
                                         Sigmoid)

            for stage in range(12):
                for wi_ in range(nw):
                    emit_stage(wi_, stage)

            nc.sync.dma_start(al_d.ap()[:], alpha_sb[:])
            nc.sync.dma_start(co_d.ap()[:], color_sb[:])

    nc.compile()
    return nc


_prog_cache = {}
_last_results = None


def _get_program(C, nslot, scales):
    key = (C, nslot, tuple(sorted(scales.items())))
    if key not in _prog_cache:
        _prog_cache[key] = _build_program(C, nslot, scales)
    return _prog_cache[key]


# ---------------------------------------------------------------------------
# Host wrapper
# ---------------------------------------------------------------------------

def kernel(**inputs):
    global _last_results
    inputs = {k: np.asarray(v) for k, v in inputs.items()}
    idx = inputs["index"].astype(np.int64)
    B = idx.shape[0]
    points = inputs["points"].astype(np.float32)
    dirs = inputs["directions"].astype(np.float32)

    # --- routing: split each expert's tokens into <=CAP_MAX chunks ("virtual
    # experts"), distribute round-robin (sorted by size) over 8 cores ---
    tok = [np.nonzero(idx == e)[0] for e in range(E)]
    virt = []  # (expert, token_ids)
    for e in range(E):
        t = tok[e]
        if len(t) == 0:
            continue
        for lo in range(0, len(t), CAP_MAX):
            virt.append((e, t[lo: lo + CAP_MAX]))
    if not virt:
        virt = [(0, np.zeros((0,), np.int64))]
    virt.sort(key=lambda v: -len(v[1]))
    nslot = max(1, int(np.ceil(len(virt) / N_CORES)))
    C = max(4, int(np.ceil(max(len(v[1]) for v in virt) / 4) * 4))
    nall = nslot * C

    core_slots = [[] for _ in range(N_CORES)]
    for i, v in enumerate(virt):
        core_slots[i % N_CORES].append(v)

    # per-layer global pow2 scales for the e3m4 stages (w5/wc0 scale over
    # the e3m4 rows only; their fp16 skip/rays slabs reuse the same k)
    scales = {}
    for st, name in ((1, "w1"), (2, "w2"), (3, "w3"), (4, "w4"),
                     (6, "w6"), (7, "w7"), (8, "wi")):
        gmax = float(np.abs(inputs[name]).max())
        scales[st] = float(np.floor(np.log2(TARGET / gmax)))
    scales[5] = float(np.floor(np.log2(
        TARGET / float(np.abs(inputs["w5"][:, 0:256]).max()))))
    scales[10] = float(np.floor(np.log2(
        TARGET / float(np.abs(inputs["wc0"][:, 0:256]).max()))))

    nc = _get_program(C, nslot, scales)

    # frequency expansion blocks with the pi/2 phase row (row 3):
    # pts: cols 0:18 sin, 18:36 cos; dirs: 0:12 sin, 12:24 cos
    fx2 = np.zeros((4, 36), np.float32)
    for c in range(3):
        for k in range(NX):
            fx2[c, c * NX + k] = float(2 ** k)
            fx2[c, 18 + c * NX + k] = float(2 ** k)
    fx2[3, 18:36] = PI / 2
    fd2 = np.zeros((4, 24), np.float32)
    for c in range(3):
        for k in range(ND):
            fd2[c, c * ND + k] = float(2 ** k)
            fd2[c, 12 + c * ND + k] = float(2 ** k)
    fd2[3, 12:24] = PI / 2

    e3 = ml_dtypes.float8_e3m4
    in_maps = []
    for cid in range(N_CORES):
        chunks = {"P": np.zeros((39, nslot * P_COLS), np.float32),
                  "T": np.zeros((128, nslot * TL_COLS), np.float32),
                  "E3": np.zeros((128, nslot * E3_COLS), np.float32)}
        bt = np.zeros((128, NB * nslot), np.float32)
        pd = np.zeros((4, 2 * nall + 60), np.float32)
        pd[3, 0: 2 * nall] = 1.0
        pd[:, 2 * nall: 2 * nall + 36] = fx2
        pd[:, 2 * nall + 36: 2 * nall + 60] = fd2
        xyzb = np.zeros((6, nall), np.float32)
        for s, (e, t) in enumerate(core_slots[cid]):
            _pack_expert(chunks, bt, s, nslot, inputs, e, scales)
            n = len(t)
            if n:
                pd[0:3, s * C: s * C + n] = points[t].T
                pd[0:3, nall + s * C: nall + s * C + n] = dirs[t].T
                xyzb[0:3, s * C: s * C + n] = points[t].T
                xyzb[3:6, s * C: s * C + n] = dirs[t].T
        im = {"pd": pd, "bt": bt,
              "xyzb": xyzb.astype(ml_dtypes.bfloat16),
              "wP": chunks["P"].astype(np.float16),
              "wT": chunks["T"].astype(np.float16),
              "wE3": chunks["E3"].astype(e3)}
        in_maps.append(im)

    res = run_bass_kernel_spmd(nc, in_maps, core_ids=list(range(N_CORES)))
    _last_results = res

    out = np.zeros((B, 4), np.float32)
    for cid in range(N_CORES):
        al = res.results[cid]["alpha_out"]
        co = res.results[cid]["color_out"]
        for s, (e, t) in enumerate(core_slots[cid]):
            n = len(t)
            if n:
                out[t, 0] = al[0, s * C: s * C + n]
                out[t, 1:4] = co[:, s * C: s * C + n].T
    return out


# revision 11
# speedup vs baseline: 1.1328x; 1.0203x over previous
"""NerfExperts MoE kernel for Trainium2, expert-parallel over 8 NeuronCores.

Each of the 1024 points is routed to one of 100 experts (~2.3MB fp32 of
weights each -> memory bound).  Experts are sharded across the 8 cores
(~13 slots/core); tokens are dispatched host-side; each expert's weights
stream from HBM exactly once in mixed precision:

  * w1-w7(mid), wi, wc0(inter) -> fp8 e3m4, scaled by a per-layer power
    of two (max |W| ~ 7, e3m4's normal band); the 2^-k descale rides the
    PSUM->SBUF move.  The fp16 skip/rays slabs that accumulate into the
    same PSUM groups are pre-scaled by the same 2^k (exact in fp16).
  * w0, w5skip, wc0rays, wa, wc1 -> fp16.  biases fp32.  Activations
    stay bf16 (mixed-dtype matmul is legal; only fp32 must match).

Weight DMA is ~8.0MB per core, streamed over the two HWDGE rings only
(SWDGE/gpsimd throttles every SDMA engine's packet rate by ~33%).  The
e3m4 mega tensor goes as per-couple-of-layers column slices so the
arrival curve tracks the stage-by-stage consumption curve.

The per-stage dependency chain avoids the ACT engine entirely:
relu(psum + b) is computed on DVE as (psum * 2^-k  MAX  -b) ADD b (two
DVE ops, no cross-engine hop).  ACT only runs the prologue Sin, the
final Sigmoid, and two late DMA kicks.

Harmonic embedding: one fused chain per source -- the frequency matmul
carries a constant pi/2 "phase row" (ones row in the points tensor x a
[0 | pi/2] row in the freq matrix), so sin rows 0:18 and cos rows 18:36
come out of a single range-reduction + Sin pass.  Token columns are
packed per-wave with a per-wave capacity C_w (max expert load in that
wave), cutting padded-column compute by ~25%.

Embedding tile rows: pts: sin 0:18, cos 18:36, xyz 36:39;
dirs: sin 0:12, cos 12:24, xyz 24:27.
"""

import numpy as np
import ml_dtypes

import concourse.bass as bass
import concourse.bacc as bacc
import concourse.mybir as mybir
import concourse.tile as tile
from concourse.bass_utils import run_bass_kernel_spmd

PI = float(np.pi)
N_CORES = 8
E = 100
NX, ND = 6, 4
CAP_MAX = 128  # max tokens per expert slot
TARGET = 10.0  # e3m4 scale target for max|W| (normal band, <15.5)

# stages with e3m4 weights (descale 2^-k in the PSUM->SBUF move)
E3_STAGES = (1, 2, 3, 4, 5, 6, 7, 8)
E3_COLS = 8 * 512 + 256          # per-slot columns in the e3m4 mega tensor

# fp32 bias tensor [128, 2*NB*nslot]: first NB*nslot cols as below, then
# the same columns negated (for the DVE relu max-trick).
#   mlp stage lidx in 0..8 (layers 0-7, then wi): col = lidx*2*nslot + s*2 + j
#   ba: 18*nslot + s ; bc0: 19*nslot + s ; bc1: 20*nslot + s
NB = 21

P_COLS = 640   # per slot: w0 256 | w5skip 256 | rays 128  (39 rows, fp16)
TL_COLS = 5    # per slot: wa 2 | wc1 3                    (128 rows, fp16)


def _e3_off(st, nslot):
    # column offset of stage block in the e3m4 mega tensor
    # st in 1..7 -> mid layers; 8 -> wi; 10 -> wc0 inter
    if st == 10:
        return 8 * 512 * nslot
    return (st - 1) * 512 * nslot


def _pack_expert(chunks, bt, s, nslot, inputs, e, scales):
    """Fill slot s of the per-chunk host arrays (fp32; cast later)."""
    n2 = 2 * nslot

    def set_b2(lidx, b):
        bt[:, lidx * n2 + s * 2] = b[0:128]
        bt[:, lidx * n2 + s * 2 + 1] = b[128:256]

    # --- P chunk (39 rows): w0 | w5 skip part | wc0 rays part.  skip and
    # rays are pre-scaled by their stage's 2^k so the shared PSUM
    # accumulation with the e3m4 slabs stays uniform. ---
    p = chunks["P"]
    o = s * P_COLS
    p[0:39, o: o + 256] = inputs["w0"][e]                 # [39, 256]
    p[0:39, o + 256: o + 512] = inputs["w5"][e][256:295] * 2.0 ** scales[5]
    p[0:27, o + 512: o + 640] = inputs["wc0"][e][256:283] * 2.0 ** scales[10]
    set_b2(0, inputs["b0"][e])
    # --- e3m4 mega tensor: 512-col blocks (k-split halves of [256,256]) ---
    w8 = chunks["E3"]
    for st, name in ((1, "w1"), (2, "w2"), (3, "w3"), (4, "w4"),
                     (5, "w5"), (6, "w6"), (7, "w7"), (8, "wi")):
        w = inputs[name][e][0:256]
        sc = 2.0 ** scales[st]
        o = _e3_off(st, nslot) + s * 512
        for k in (0, 1):
            w8[:, o + k * 256: o + (k + 1) * 256] = \
                w[128 * k: 128 * (k + 1)] * sc
        set_b2(st, inputs["b5" if name == "w5" else
                          ("bi" if name == "wi" else f"b{st}")][e])
    wc0 = inputs["wc0"][e]
    o = _e3_off(10, nslot) + s * 256
    sc = 2.0 ** scales[10]
    w8[:, o: o + 128] = wc0[0:128] * sc
    w8[:, o + 128: o + 256] = wc0[128:256] * sc
    bt[:, 19 * nslot + s] = inputs["bc0"][e]
    # --- tiny fp16 tail: wa | wc1 ---
    t = chunks["T"]
    o = s * TL_COLS
    wa = inputs["wa"][e][:, 0]
    t[:, o] = wa[0:128]
    t[:, o + 1] = wa[128:256]
    bt[0, 18 * nslot + s] = inputs["ba"][e][0]
    t[:, o + 2: o + 5] = inputs["wc1"][e]
    bt[0:3, 20 * nslot + s] = inputs["bc1"][e]


def _make_waves(nslot, slot_cap):
    """Wave list [(s0, s1, Cw, off)] with per-wave capacity."""
    Cmax = max(4, int(np.ceil(max(slot_cap) / 4) * 4))
    gmax = max(1, min(512 // (2 * Cmax), 6))
    nw = int(np.ceil(nslot / gmax))
    base = nslot // nw
    rem = nslot - base * nw
    sizes = [base + (1 if i < rem else 0) for i in range(nw)]
    waves, s0, off = [], 0, 0
    for g in sizes:
        cw = max(4, int(np.ceil(max(slot_cap[s0:s0 + g]) / 4) * 4))
        waves.append((s0, s0 + g, cw, off))
        off += g * cw
        s0 += g
    return waves


# ---------------------------------------------------------------------------
# Device program
# ---------------------------------------------------------------------------

def _build_program(waves, nslot, scales):
    """SPMD Bass program: nslot expert slots, per-wave token capacity."""
    waves = list(waves)
    nall = sum((s1 - s0) * cw for s0, s1, cw, _ in waves)
    nw = len(waves)
    f32 = mybir.dt.float32
    f16 = mybir.dt.float16
    bf16 = mybir.dt.bfloat16
    f8e3 = mybir.dt.float8e3
    Sin = mybir.ActivationFunctionType.Sin
    Sigmoid = mybir.ActivationFunctionType.Sigmoid
    ADD = mybir.AluOpType.add
    SUB = mybir.AluOpType.subtract
    MUL = mybir.AluOpType.mult
    MAX = mybir.AluOpType.max
    MIN = mybir.AluOpType.min
    # range-reduction constants (Cody-Waite, fp32 magic rounding)
    INV2PI = float(np.float32(1.0 / (2 * PI)))
    MAGIC = 12582912.0            # 1.5 * 2**23: forces round-to-int in fp32
    C1 = 6.28125                  # 2*pi high part, exact in fp32
    C2 = float(np.float32(2 * PI - 6.28125))
    CLAMP = 3.1415925             # just under pi (ACT Sin domain is [-pi, pi])

    nc = bacc.Bacc("TRN2", target_bir_lowering=False, debug=False)
    pd_d = nc.dram_tensor("pd", (4, 2 * nall + 60), f32, kind="ExternalInput")
    xyz_d = nc.dram_tensor("xyzb", (6, nall), bf16, kind="ExternalInput")
    bt_d = nc.dram_tensor("bt", (128, 2 * NB * nslot), f32,
                          kind="ExternalInput")
    wp_d = nc.dram_tensor("wP", (39, nslot * P_COLS), f16, kind="ExternalInput")
    we3_d = nc.dram_tensor("wE3", (128, nslot * E3_COLS), f8e3,
                           kind="ExternalInput")
    wt_d = nc.dram_tensor("wT", (128, nslot * TL_COLS), f16,
                          kind="ExternalInput")
    al_d = nc.dram_tensor("alpha_out", (1, nall), f32, kind="ExternalOutput")
    co_d = nc.dram_tensor("color_out", (3, nall), f32, kind="ExternalOutput")

    with tile.TileContext(nc) as tc:
        with (
            tc.tile_pool(name="cp", bufs=1) as cp,
            tc.tile_pool(name="xp", bufs=2 * nw + 2) as xp,
            tc.tile_pool(name="psA", bufs=7, space=bass.MemorySpace.PSUM) as psA,
            tc.tile_pool(name="psB", bufs=1, space=bass.MemorySpace.PSUM) as psB,
        ):
            embP = cp.tile([39, nall], bf16)  # sin 0:18, cos 18:36, xyz 36:39
            embD = cp.tile([27, nall], bf16)  # sin 0:12, cos 12:24, xyz 24:27
            pd_sb = cp.tile([4, 2 * nall + 60], f32)
            bt_sb = cp.tile([128, 2 * NB * nslot], f32)
            wp_sb = cp.tile([39, nslot * P_COLS], f16, name="wP", tag="wP")
            we3_sb = cp.tile([128, nslot * E3_COLS], f8e3, name="wE3",
                             tag="wE3")
            wt_sb = cp.tile([128, nslot * TL_COLS], f16, name="wT", tag="wT")

            # ---- DMA kicks, all before any compute so every ring's kicks
            # precede its compute work.  sync carries the early/mid chunks
            # (its ring is compute-free); scalar carries only the two late
            # slices, keeping the prologue Sin unblocked. ----
            nc.sync.dma_start(pd_sb[:], pd_d.ap()[:])
            nc.sync.dma_start(embP[36:39, :], xyz_d.ap()[0:3, :])
            nc.sync.dma_start(embD[24:27, :], xyz_d.ap()[3:6, :])
            nc.sync.dma_start(bt_sb[:], bt_d.ap()[:])
            nc.sync.dma_start(wt_sb[:], wt_d.ap()[:])
            nc.sync.dma_start(wp_sb[:], wp_d.ap()[:])
            cuts = [0, _e3_off(2, nslot), _e3_off(4, nslot),
                    _e3_off(6, nslot), _e3_off(8, nslot), nslot * E3_COLS]
            # slices: (w1), (w2,w3), (w4,w5) on sync; (w6,w7), (wi,wc0i)
            # on scalar
            for i in range(3):
                nc.sync.dma_start(we3_sb[:, cuts[i]:cuts[i + 1]],
                                  we3_d.ap()[:, cuts[i]:cuts[i + 1]])
            for i in range(3, 5):
                nc.scalar.dma_start(we3_sb[:, cuts[i]:cuts[i + 1]],
                                    we3_d.ap()[:, cuts[i]:cuts[i + 1]])

            alpha_sb = cp.tile([1, nall], f32)
            color_sb = cp.tile([3, nall], f32)

            # ---- harmonic embedding: ONE fused sin+cos chain per source.
            # ep rows 0:half are freq*x, rows half:2*half are freq*x + pi/2
            # (phase row trick), so Sin() yields sin|cos in one shot. ----
            def reduce_sin(tsrc, rows, ncol):
                t1 = xp.tile([rows, ncol], f32, tag="vred")
                nc.vector.tensor_scalar(t1[:], tsrc, INV2PI, MAGIC, MUL, ADD)
                r = xp.tile([rows, ncol], f32, tag="vred")
                nc.vector.tensor_scalar(r[:], t1[:], MAGIC, None, SUB)
                rd = xp.tile([rows, ncol], f32, tag="vred")
                nc.vector.scalar_tensor_tensor(rd[:], r[:], -C1, tsrc, MUL, ADD)
                rd2 = xp.tile([rows, ncol], f32, tag="vred")
                nc.vector.scalar_tensor_tensor(rd2[:], r[:], -C2, rd[:], MUL, ADD)
                v = xp.tile([rows, ncol], f32, tag="vred")
                nc.vector.tensor_scalar(v[:], rd2[:], CLAMP, -CLAMP, MIN, MAX)
                return v

            for lo in range(0, nall, 512):
                hi = min(nall, lo + 512)
                w_ = hi - lo
                for (rows, fcol, src_lo, dst) in (
                    (36, 2 * nall, 0, embP),
                    (24, 2 * nall + 36, nall, embD),
                ):
                    ep = psA.tile([rows, w_], f32, tag="mlp")
                    nc.tensor.matmul(ep[:], pd_sb[0:4, fcol: fcol + rows],
                                     pd_sb[0:4, src_lo + lo: src_lo + hi],
                                     start=True, stop=True)
                    vs = reduce_sin(ep[:], rows, w_)
                    nc.scalar.activation(dst[0:rows, lo:hi], vs[:], Sin)

            # ---- wave-lockstep MLP.  relu(psum+b) is computed on DVE as
            # (psum*dsc MAX -b) ADD b -- no ACT hop in the chain. ----
            NEG = NB * nslot  # column offset of the negated bias copy

            def bias2(lidx, s0, s1, neg=False):
                base = (NEG if neg else 0) + lidx * 2 * nslot
                return bt_sb[:, base + s0 * 2: base + s1 * 2]

            xs = [None] * nw
            its = [None] * nw
            cts = [None] * nw

            def slab(st, s, lo, hi):
                o = _e3_off(st, nslot) + s * (256 if st == 10 else 512)
                return we3_sb[0:128, o + lo: o + hi]

            def emit_stage(wi_, stage):
                s0, s1, C, off = waves[wi_]
                g = s1 - s0

                def xsl(t, i, j):
                    return t[:, (2 * i + j) * C:(2 * i + j + 1) * C]

                def tok(i):
                    return slice(off + i * C, off + (i + 1) * C)

                def move2(ps, lidx, relu=True):
                    dsc = float(2.0 ** (-scales[lidx])) \
                        if lidx in E3_STAGES else 1.0
                    xn = xp.tile([128, 2 * g * C], bf16, tag="x")
                    psv = ps[:].rearrange("p (a c) -> p a c", a=2 * g)
                    xnv = xn[:].rearrange("p (a c) -> p a c", a=2 * g)
                    bpos = bias2(lidx, s0, s1).broadcast_to([128, 2 * g, C])
                    if relu:
                        bneg = bias2(lidx, s0, s1, neg=True).broadcast_to(
                            [128, 2 * g, C])
                        tm = xp.tile([128, 2 * g * C], f32, tag="tm")
                        tmv = tm[:].rearrange("p (a c) -> p a c", a=2 * g)
                        nc.vector.scalar_tensor_tensor(
                            tmv, psv, dsc, bneg, MUL, MAX)
                        nc.vector.tensor_tensor(xnv, tmv, bpos, ADD)
                    else:
                        nc.vector.scalar_tensor_tensor(
                            xnv, psv, dsc, bpos, MUL, ADD)
                    return xn

                if stage == 0:  # w0 (fp16, 39-row slabs)
                    ps = psA.tile([128, 2 * g * C], f32, tag="mlp")
                    for i in range(g):
                        s = s0 + i
                        o = s * P_COLS
                        for j in (0, 1):
                            nc.tensor.matmul(
                                xsl(ps, i, j),
                                wp_sb[0:39, o + j * 128: o + j * 128 + 128],
                                embP[0:39, tok(i)], start=True, stop=True)
                    xs[wi_] = move2(ps, 0)
                elif stage in (1, 2, 3, 4, 6, 7):
                    ps = psA.tile([128, 2 * g * C], f32, tag="mlp")
                    xin = xs[wi_]
                    for i in range(g):
                        s = s0 + i
                        for j in (0, 1):
                            pj = xsl(ps, i, j)
                            nc.tensor.matmul(
                                pj, slab(stage, s, j * 128, j * 128 + 128),
                                xsl(xin, i, 0), start=True, stop=False)
                            nc.tensor.matmul(
                                pj, slab(stage, s, 256 + j * 128, 384 + j * 128),
                                xsl(xin, i, 1), start=False, stop=True)
                    xs[wi_] = move2(ps, stage)
                elif stage == 5:  # e3m4 mid + pre-scaled fp16 skip slab
                    ps = psA.tile([128, 2 * g * C], f32, tag="mlp")
                    xin = xs[wi_]
                    for i in range(g):
                        s = s0 + i
                        o = s * P_COLS
                        for j in (0, 1):
                            pj = xsl(ps, i, j)
                            nc.tensor.matmul(pj, slab(5, s, j * 128, j * 128 + 128),
                                             xsl(xin, i, 0),
                                             start=True, stop=False)
                            nc.tensor.matmul(pj, slab(5, s, 256 + j * 128, 384 + j * 128),
                                             xsl(xin, i, 1),
                                             start=False, stop=False)
                            nc.tensor.matmul(
                                pj,
                                wp_sb[0:39, o + 256 + j * 128: o + 384 + j * 128],
                                embP[0:39, tok(i)], start=False, stop=True)
                    xs[wi_] = move2(ps, 5)
                elif stage == 8:  # wi -> inter (e3m4; bias, no relu)
                    ps = psA.tile([128, 2 * g * C], f32, tag="mlp")
                    xin = xs[wi_]
                    for i in range(g):
                        s = s0 + i
                        for j in (0, 1):
                            pj = xsl(ps, i, j)
                            nc.tensor.matmul(pj, slab(8, s, j * 128, j * 128 + 128),
                                             xsl(xin, i, 0),
                                             start=True, stop=False)
                            nc.tensor.matmul(pj, slab(8, s, 256 + j * 128, 384 + j * 128),
                                             xsl(xin, i, 1),
                                             start=False, stop=True)
                    its[wi_] = move2(ps, 8, relu=False)
                elif stage == 9:  # wa -> alpha (fp16, tail chunk)
                    pa = psB.tile([3, g * C], f32, tag="head")
                    xin = xs[wi_]
                    for i in range(g):
                        s = s0 + i
                        o = s * TL_COLS
                        nc.tensor.matmul(pa[0:1, i * C:(i + 1) * C],
                                         wt_sb[:, o: o + 1],
                                         xsl(xin, i, 0),
                                         start=True, stop=False)
                        nc.tensor.matmul(pa[0:1, i * C:(i + 1) * C],
                                         wt_sb[:, o + 1: o + 2],
                                         xsl(xin, i, 1),
                                         start=False, stop=True)
                    av = alpha_sb[0:1, off: off + g * C].rearrange(
                        "p (g c) -> p g c", g=g)
                    pav = pa[0:1, :].rearrange("p (g c) -> p g c", g=g)
                    nc.vector.tensor_tensor(
                        av, pav,
                        bt_sb[0:1, 18 * nslot + s0: 18 * nslot + s1]
                        .broadcast_to([1, g, C]), ADD)
                elif stage == 10:  # wc0 (e3m4 inter + pre-scaled fp16 rays)
                    pc = psA.tile([128, g * C], f32, tag="mlp")
                    it = its[wi_]
                    for i in range(g):
                        s = s0 + i
                        op = s * P_COLS
                        pj = pc[:, i * C:(i + 1) * C]
                        nc.tensor.matmul(pj, slab(10, s, 0, 128),
                                         xsl(it, i, 0),
                                         start=True, stop=False)
                        nc.tensor.matmul(pj, slab(10, s, 128, 256),
                                         xsl(it, i, 1),
                                         start=False, stop=False)
                        nc.tensor.matmul(pj, wp_sb[0:27, op + 512: op + 640],
                                         embD[0:27, tok(i)],
                                         start=False, stop=True)
                    dsc = float(2.0 ** (-scales[10]))
                    ct = xp.tile([128, g * C], bf16, tag="ct")
                    tm = xp.tile([128, g * C], f32, tag="tm")
                    pcv = pc[:].rearrange("p (g c) -> p g c", g=g)
                    tmv = tm[:].rearrange("p (g c) -> p g c", g=g)
                    ctv = ct[:].rearrange("p (g c) -> p g c", g=g)
                    bpos = bt_sb[:, 19 * nslot + s0: 19 * nslot + s1] \
                        .broadcast_to([128, g, C])
                    bneg = bt_sb[:, NEG + 19 * nslot + s0: NEG + 19 * nslot + s1] \
                        .broadcast_to([128, g, C])
                    nc.vector.scalar_tensor_tensor(tmv, pcv, dsc, bneg, MUL, MAX)
                    nc.vector.tensor_tensor(ctv, tmv, bpos, ADD)
                    cts[wi_] = ct
                elif stage == 11:  # wc1 -> sigmoid color (fp16, tail chunk)
                    pcol = psB.tile([3, g * C], f32, tag="head")
                    ct = cts[wi_]
                    for i in range(g):
                        s = s0 + i
                        o = s * TL_COLS
                        nc.tensor.matmul(pcol[:, i * C:(i + 1) * C],
                                         wt_sb[:, o + 2: o + 5],
                                         ct[:, i * C:(i + 1) * C],
                                         start=True, stop=True)
                    ctmp = xp.tile([3, g * C], f32, tag="ctmp")
                    pv = pcol[:].rearrange("p (g c) -> p g c", g=g)
                    cv = ctmp[:].rearrange("p (g c) -> p g c", g=g)
                    nc.vector.tensor_tensor(
                        cv, pv,
                        bt_sb[0:3, 20 * nslot + s0: 20 * nslot + s1]
                        .broadcast_to([3, g, C]), ADD)
                    nc.scalar.activation(color_sb[0:3, off: off + g * C],
                                         ctmp[:], Sigmoid)

            for stage in range(12):
                for wi_ in range(nw):
                    emit_stage(wi_, stage)

            nc.sync.dma_start(al_d.ap()[:], alpha_sb[:])
            nc.sync.dma_start(co_d.ap()[:], color_sb[:])

    nc.compile()
    return nc


_prog_cache = {}
_last_results = None


def _get_program(waves, nslot, scales):
    key = (tuple(waves), nslot, tuple(sorted(scales.items())))
    if key not in _prog_cache:
        _prog_cache[key] = _build_program(waves, nslot, scales)
    return _prog_cache[key]


# ---------------------------------------------------------------------------
# Host wrapper
# ---------------------------------------------------------------------------

def kernel(**inputs):
    global _last_results
    inputs = {k: np.asarray(v) for k, v in inputs.items()}
    idx = inputs["index"].astype(np.int64)
    B = idx.shape[0]
    points = inputs["points"].astype(np.float32)
    dirs = inputs["directions"].astype(np.float32)

    # --- routing: split each expert's tokens into <=CAP_MAX chunks ("virtual
    # experts"), distribute round-robin (sorted by size) over 8 cores ---
    tok = [np.nonzero(idx == e)[0] for e in range(E)]
    virt = []  # (expert, token_ids)
    for e in range(E):
        t = tok[e]
        if len(t) == 0:
            continue
        for lo in range(0, len(t), CAP_MAX):
            virt.append((e, t[lo: lo + CAP_MAX]))
    if not virt:
        virt = [(0, np.zeros((0,), np.int64))]
    virt.sort(key=lambda v: -len(v[1]))
    nslot = max(1, int(np.ceil(len(virt) / N_CORES)))

    core_slots = [[] for _ in range(N_CORES)]
    for i, v in enumerate(virt):
        core_slots[i % N_CORES].append(v)

    # per-slot-index capacity = max token count across cores (sorted deal
    # means slot i's max is virt[8i])
    slot_cap = [len(virt[N_CORES * i][1]) if N_CORES * i < len(virt) else 1
                for i in range(nslot)]
    waves = _make_waves(nslot, slot_cap)
    nall = sum((s1 - s0) * cw for s0, s1, cw, _ in waves)
    # slot -> (column offset, capacity)
    slot_pos = {}
    for s0, s1, cw, off in waves:
        for i in range(s1 - s0):
            slot_pos[s0 + i] = (off + i * cw, cw)

    # per-layer global pow2 scales for the e3m4 stages (w5/wc0 scale over
    # the e3m4 rows only; their fp16 skip/rays slabs reuse the same k)
    scales = {}
    for st, name in ((1, "w1"), (2, "w2"), (3, "w3"), (4, "w4"),
                     (6, "w6"), (7, "w7"), (8, "wi")):
        gmax = float(np.abs(inputs[name]).max())
        scales[st] = float(np.floor(np.log2(TARGET / gmax)))
    scales[5] = float(np.floor(np.log2(
        TARGET / float(np.abs(inputs["w5"][:, 0:256]).max()))))
    scales[10] = float(np.floor(np.log2(
        TARGET / float(np.abs(inputs["wc0"][:, 0:256]).max()))))

    nc = _get_program(waves, nslot, scales)

    # frequency expansion blocks with the pi/2 phase row (row 3):
    # pts: cols 0:18 sin, 18:36 cos; dirs: 0:12 sin, 12:24 cos
    fx2 = np.zeros((4, 36), np.float32)
    for c in range(3):
        for k in range(NX):
            fx2[c, c * NX + k] = float(2 ** k)
            fx2[c, 18 + c * NX + k] = float(2 ** k)
    fx2[3, 18:36] = PI / 2
    fd2 = np.zeros((4, 24), np.float32)
    for c in range(3):
        for k in range(ND):
            fd2[c, c * ND + k] = float(2 ** k)
            fd2[c, 12 + c * ND + k] = float(2 ** k)
    fd2[3, 12:24] = PI / 2

    e3 = ml_dtypes.float8_e3m4
    in_maps = []
    for cid in range(N_CORES):
        chunks = {"P": np.zeros((39, nslot * P_COLS), np.float32),
                  "T": np.zeros((128, nslot * TL_COLS), np.float32),
                  "E3": np.zeros((128, nslot * E3_COLS), np.float32)}
        bt = np.zeros((128, NB * nslot), np.float32)
        pd = np.zeros((4, 2 * nall + 60), np.float32)
        pd[3, 0: 2 * nall] = 1.0
        pd[:, 2 * nall: 2 * nall + 36] = fx2
        pd[:, 2 * nall + 36: 2 * nall + 60] = fd2
        xyzb = np.zeros((6, nall), np.float32)
        for s, (e, t) in enumerate(core_slots[cid]):
            _pack_expert(chunks, bt, s, nslot, inputs, e, scales)
            n = len(t)
            if n:
                o, _ = slot_pos[s]
                pd[0:3, o: o + n] = points[t].T
                pd[0:3, nall + o: nall + o + n] = dirs[t].T
                xyzb[0:3, o: o + n] = points[t].T
                xyzb[3:6, o: o + n] = dirs[t].T
        im = {"pd": pd,
              "bt": np.concatenate([bt, -bt], axis=1),
              "xyzb": xyzb.astype(ml_dtypes.bfloat16),
              "wP": chunks["P"].astype(np.float16),
              "wT": chunks["T"].astype(np.float16),
              "wE3": chunks["E3"].astype(e3)}
        in_maps.append(im)

    res = run_bass_kernel_spmd(nc, in_maps, core_ids=list(range(N_CORES)))
    _last_results = res

    out = np.zeros((B, 4), np.float32)
    for cid in range(N_CORES):
        al = res.results[cid]["alpha_out"]
        co = res.results[cid]["color_out"]
        for s, (e, t) in enumerate(core_slots[cid]):
            n = len(t)
            if n:
                o, _ = slot_pos[s]
                out[t, 0] = al[0, o: o + n]
                out[t, 1:4] = co[:, o: o + n].T
    return out


# revision 14
# speedup vs baseline: 1.1635x; 1.0271x over previous
"""NerfExperts MoE kernel for Trainium2, expert-parallel over 8 NeuronCores.

Each of the 1024 points is routed to one of 100 experts (~2.3MB fp32 of
weights each -> memory bound).  Experts are sharded across the 8 cores
(~13 slots/core); tokens are dispatched host-side; each expert's weights
stream from HBM exactly once in mixed precision:

  * w1-w7(mid), wi, wc0(inter) -> fp8 e3m4, scaled by a per-layer power
    of two (max |W| ~ 7, e3m4's normal band); the 2^-k descale rides the
    PSUM->SBUF move.  The fp16 skip/rays slabs that accumulate into the
    same PSUM groups are pre-scaled by the same 2^k (exact in fp16).
  * w0, w5skip, wc0rays, wa, wc1 -> fp16.  biases fp32.  Activations
    stay bf16 (mixed-dtype matmul is legal; only fp32 must match).

Weight DMA is ~8.0MB per core, streamed over the two HWDGE rings only
(SWDGE/gpsimd throttles every SDMA engine's packet rate by ~33%).  The
e3m4 mega tensor goes as per-couple-of-layers column slices so the
arrival curve tracks the stage-by-stage consumption curve.

The per-stage dependency chain is PE -> one DVE op -> PE: the layer
output is stored SHIFTED as t = max(psum * 2^-k, -B) (= relu(z) - B),
and the missing "+B" is folded into the NEXT layer's bias on the host
(B_{l+1} = b_{l+1} + Wq_{l+1}^T B_l, computed with the exact quantized
weights).  No second DVE op, no ACT relu, no cross-engine hop.  sin is
evaluated as a degree-7 odd polynomial ON DVE after Cody-Waite range
reduction, so the ACT engine runs ONLY the final three Sigmoids -- its
HWDGE ring is a pure DMA courier otherwise.

Harmonic embedding: one fused chain per source -- the frequency matmul
carries a constant pi/2 "phase row" (ones row in the points tensor x a
[0 | pi/2] row in the freq matrix), so sin rows 0:18 and cos rows 18:36
come out of a single range-reduction + poly pass.  Token columns are
packed per-wave with a per-wave capacity C_w (max expert load in that
wave), cutting padded-column compute by ~25%.

Embedding tile rows: pts: sin 0:18, cos 18:36, xyz 36:39;
dirs: sin 0:12, cos 12:24, xyz 24:27.
"""

import numpy as np
import ml_dtypes

import concourse.bass as bass
import concourse.bacc as bacc
import concourse.mybir as mybir
import concourse.tile as tile
from concourse.bass_utils import run_bass_kernel_spmd

PI = float(np.pi)
N_CORES = 8
E = 100
NX, ND = 6, 4
CAP_MAX = 128  # max tokens per expert slot
TARGET = 10.0  # e3m4 scale target for max|W| (normal band, <15.5)

# stages with e3m4 weights (descale 2^-k in the PSUM->SBUF move)
E3_STAGES = (1, 2, 3, 4, 5, 6, 7, 8)
E3_COLS = 8 * 512 + 256          # per-slot columns in the e3m4 mega tensor

# fp32 bias tensor [128, 2*NB*nslot]: first NB*nslot cols as below, then
# the same columns negated (for the DVE relu max-trick).
#   mlp stage lidx in 0..8 (layers 0-7, then wi): col = lidx*2*nslot + s*2 + j
#   ba: 18*nslot + s ; bc0: 19*nslot + s ; bc1: 20*nslot + s
NB = 21

P_COLS = 640   # per slot: w0 256 | w5skip 256 | rays 128  (39 rows, fp16)
TL_COLS = 5    # per slot: wa 2 | wc1 3                    (128 rows, fp16)


def _e3_off(st, nslot):
    # column offset of stage block in the e3m4 mega tensor
    # st in 1..7 -> mid layers; 8 -> wi; 10 -> wc0 inter
    if st == 10:
        return 8 * 512 * nslot
    return (st - 1) * 512 * nslot


def _pack_expert(chunks, bt, s, nslot, inputs, e, scales, WS, FB):
    """Fill slot s of the per-chunk host arrays (fp32; cast later).
    WS[name] holds the device-exact scaled weights (quantize-idempotent
    for e3m4 names); FB holds the folded biases."""
    n2 = 2 * nslot

    def set_b2(lidx, b):
        bt[:, lidx * n2 + s * 2] = b[0:128]
        bt[:, lidx * n2 + s * 2 + 1] = b[128:256]

    # --- P chunk (39 rows): w0 | w5 skip part | wc0 rays part.  skip and
    # rays are pre-scaled by their stage's 2^k so the shared PSUM
    # accumulation with the e3m4 slabs stays uniform. ---
    p = chunks["P"]
    o = s * P_COLS
    p[0:39, o: o + 256] = inputs["w0"][e]                 # [39, 256]
    p[0:39, o + 256: o + 512] = inputs["w5"][e][256:295] * 2.0 ** scales[5]
    p[0:27, o + 512: o + 640] = inputs["wc0"][e][256:283] * 2.0 ** scales[10]
    set_b2(0, FB["b0"][e])
    # --- e3m4 mega tensor: 512-col blocks (k-split halves of [256,256]) ---
    w8 = chunks["E3"]
    for st, name in ((1, "w1"), (2, "w2"), (3, "w3"), (4, "w4"),
                     (5, "w5"), (6, "w6"), (7, "w7"), (8, "wi")):
        w = WS[name][e]
        o = _e3_off(st, nslot) + s * 512
        for k in (0, 1):
            w8[:, o + k * 256: o + (k + 1) * 256] = w[128 * k: 128 * (k + 1)]
        set_b2(st, FB["b5" if name == "w5" else
                   ("bi" if name == "wi" else f"b{st}")][e])
    o = _e3_off(10, nslot) + s * 256
    wc0s = WS["wc0"][e]
    w8[:, o: o + 128] = wc0s[0:128]
    w8[:, o + 128: o + 256] = wc0s[128:256]
    bt[:, 19 * nslot + s] = inputs["bc0"][e]
    # --- tiny fp16 tail: wa | wc1 ---
    t = chunks["T"]
    o = s * TL_COLS
    wa = inputs["wa"][e][:, 0]
    t[:, o] = wa[0:128]
    t[:, o + 1] = wa[128:256]
    bt[0, 18 * nslot + s] = FB["ba"][e][0]
    t[:, o + 2: o + 5] = inputs["wc1"][e]
    bt[0:3, 20 * nslot + s] = FB["bc1"][e]


def _make_waves(nslot, slot_cap):
    """Wave list [(s0, s1, Cw, off)] with per-wave capacity."""
    Cmax = max(4, int(np.ceil(max(slot_cap) / 4) * 4))
    gmax = max(1, min(512 // (2 * Cmax), 6))
    nw = int(np.ceil(nslot / gmax))
    base = nslot // nw
    rem = nslot - base * nw
    sizes = [base + (1 if i < rem else 0) for i in range(nw)]
    waves, s0, off = [], 0, 0
    for g in sizes:
        cw = max(4, int(np.ceil(max(slot_cap[s0:s0 + g]) / 4) * 4))
        waves.append((s0, s0 + g, cw, off))
        off += g * cw
        s0 += g
    return waves


# ---------------------------------------------------------------------------
# Device program
# ---------------------------------------------------------------------------

def _build_program(waves, nslot, scales):
    """SPMD Bass program: nslot expert slots, per-wave token capacity."""
    waves = list(waves)
    nall = sum((s1 - s0) * cw for s0, s1, cw, _ in waves)
    nw = len(waves)
    f32 = mybir.dt.float32
    f16 = mybir.dt.float16
    bf16 = mybir.dt.bfloat16
    f8e3 = mybir.dt.float8e3
    Sin = mybir.ActivationFunctionType.Sin
    Sigmoid = mybir.ActivationFunctionType.Sigmoid
    ADD = mybir.AluOpType.add
    SUB = mybir.AluOpType.subtract
    MUL = mybir.AluOpType.mult
    MAX = mybir.AluOpType.max
    MIN = mybir.AluOpType.min
    # range-reduction constants (Cody-Waite, fp32 magic rounding)
    INV2PI = float(np.float32(1.0 / (2 * PI)))
    MAGIC = 12582912.0            # 1.5 * 2**23: forces round-to-int in fp32
    C1 = 6.28125                  # 2*pi high part, exact in fp32
    C2 = float(np.float32(2 * PI - 6.28125))
    CLAMP = 3.1415925             # just under pi (ACT Sin domain is [-pi, pi])

    nc = bacc.Bacc("TRN2", target_bir_lowering=False, debug=False)
    pd_d = nc.dram_tensor("pd", (4, 2 * nall + 60), f32, kind="ExternalInput")
    xyz_d = nc.dram_tensor("xyzb", (6, nall), bf16, kind="ExternalInput")
    bt_d = nc.dram_tensor("bt", (128, 2 * NB * nslot), f32,
                          kind="ExternalInput")
    wp_d = nc.dram_tensor("wP", (39, nslot * P_COLS), f16, kind="ExternalInput")
    we3_d = nc.dram_tensor("wE3", (128, nslot * E3_COLS), f8e3,
                           kind="ExternalInput")
    wt_d = nc.dram_tensor("wT", (128, nslot * TL_COLS), f16,
                          kind="ExternalInput")
    al_d = nc.dram_tensor("alpha_out", (1, nall), f32, kind="ExternalOutput")
    co_d = nc.dram_tensor("color_out", (3, nall), f32, kind="ExternalOutput")

    with tile.TileContext(nc) as tc:
        with (
            tc.tile_pool(name="cp", bufs=1) as cp,
            tc.tile_pool(name="xp", bufs=2 * nw + 2) as xp,
            tc.tile_pool(name="psA", bufs=7, space=bass.MemorySpace.PSUM) as psA,
            tc.tile_pool(name="psB", bufs=1, space=bass.MemorySpace.PSUM) as psB,
        ):
            embP = cp.tile([39, nall], bf16)  # sin 0:18, cos 18:36, xyz 36:39
            embD = cp.tile([27, nall], bf16)  # sin 0:12, cos 12:24, xyz 24:27
            pd_sb = cp.tile([4, 2 * nall + 60], f32)
            bt_sb = cp.tile([128, 2 * NB * nslot], f32)
            wp_sb = cp.tile([39, nslot * P_COLS], f16, name="wP", tag="wP")
            we3_sb = cp.tile([128, nslot * E3_COLS], f8e3, name="wE3",
                             tag="wE3")
            wt_sb = cp.tile([128, nslot * TL_COLS], f16, name="wT", tag="wT")

            # ---- DMA kicks, all before any compute.  Both HWDGE rings
            # are pure couriers now (ACT only runs the final sigmoids).
            # Per-layer e3m4 slices alternate between the rings so the
            # cumulative arrival curve matches stage consumption; the
            # small wc0i slice lands last and only gates stages 10-11. ----
            nc.sync.dma_start(pd_sb[:], pd_d.ap()[:])
            nc.sync.dma_start(bt_sb[:], bt_d.ap()[:])
            nc.scalar.dma_start(embP[36:39, :], xyz_d.ap()[0:3, :])
            nc.scalar.dma_start(embD[24:27, :], xyz_d.ap()[3:6, :])
            nc.scalar.dma_start(wt_sb[:], wt_d.ap()[:])
            nc.scalar.dma_start(wp_sb[:], wp_d.ap()[:])

            def e3_slice(eng, st):
                lo = _e3_off(st, nslot)
                hi = _e3_off(st + 1, nslot) if st < 8 else (
                    _e3_off(10, nslot) if st == 8 else nslot * E3_COLS)
                eng.dma_start(we3_sb[:, lo:hi], we3_d.ap()[:, lo:hi])

            for st in (1, 3, 5, 7):
                e3_slice(nc.sync, st)
            for st in (2, 4, 6, 8):
                e3_slice(nc.scalar, st)
            e3_slice(nc.sync, 10)

            alpha_sb = cp.tile([1, nall], f32)
            color_sb = cp.tile([3, nall], f32)

            # ---- harmonic embedding: ONE fused sin+cos chain per source.
            # ep rows 0:half are freq*x, rows half:2*half are freq*x + pi/2
            # (phase row trick).  sin is a degree-7 odd polynomial on DVE
            # after Cody-Waite range reduction -- ACT is never touched. ----
            A0, A1, A2, A3 = (9.99449986e-01, -1.65838221e-01,
                              7.99852030e-03, -1.47736456e-04)

            def poly_sin(tsrc, rows, ncol, dst_ap):
                t1 = xp.tile([rows, ncol], f32, tag="vred")
                nc.vector.tensor_scalar(t1[:], tsrc, INV2PI, MAGIC, MUL, ADD)
                r = xp.tile([rows, ncol], f32, tag="vred")
                nc.vector.tensor_scalar(r[:], t1[:], MAGIC, None, SUB)
                rd = xp.tile([rows, ncol], f32, tag="vred")
                nc.vector.scalar_tensor_tensor(rd[:], r[:], -C1, tsrc, MUL, ADD)
                v = xp.tile([rows, ncol], f32, tag="vred")
                nc.vector.scalar_tensor_tensor(v[:], r[:], -C2, rd[:], MUL, ADD)
                y2 = xp.tile([rows, ncol], f32, tag="vred")
                nc.vector.tensor_tensor(y2[:], v[:], v[:], MUL)
                h = xp.tile([rows, ncol], f32, tag="vred")
                nc.vector.tensor_scalar(h[:], y2[:], A3, A2, MUL, ADD)
                h2 = xp.tile([rows, ncol], f32, tag="vred")
                nc.vector.tensor_tensor(h2[:], h[:], y2[:], MUL)
                nc.vector.tensor_scalar(h2[:], h2[:], A1, None, ADD)
                h3 = xp.tile([rows, ncol], f32, tag="vred")
                nc.vector.tensor_tensor(h3[:], h2[:], y2[:], MUL)
                nc.vector.tensor_scalar(h3[:], h3[:], A0, None, ADD)
                nc.vector.tensor_tensor(dst_ap, h3[:], v[:], MUL)

            for lo in range(0, nall, 512):
                hi = min(nall, lo + 512)
                w_ = hi - lo
                for (rows, fcol, src_lo, dst) in (
                    (36, 2 * nall, 0, embP),
                    (24, 2 * nall + 36, nall, embD),
                ):
                    ep = psA.tile([rows, w_], f32, tag="mlp")
                    nc.tensor.matmul(ep[:], pd_sb[0:4, fcol: fcol + rows],
                                     pd_sb[0:4, src_lo + lo: src_lo + hi],
                                     start=True, stop=True)
                    poly_sin(ep[:], rows, w_, dst[0:rows, lo:hi])

            # ---- wave-lockstep MLP.  relu(psum+b) is computed on DVE as
            # (psum*dsc MAX -b) ADD b -- no ACT hop in the chain. ----
            NEG = NB * nslot  # column offset of the negated bias copy

            def bias2(lidx, s0, s1, neg=False):
                base = (NEG if neg else 0) + lidx * 2 * nslot
                return bt_sb[:, base + s0 * 2: base + s1 * 2]

            xs = [None] * nw
            its = [None] * nw
            cts = [None] * nw

            def slab(st, s, lo, hi):
                o = _e3_off(st, nslot) + s * (256 if st == 10 else 512)
                return we3_sb[0:128, o + lo: o + hi]

            def emit_stage(wi_, stage):
                s0, s1, C, off = waves[wi_]
                g = s1 - s0

                def xsl(t, i, j):
                    return t[:, (2 * i + j) * C:(2 * i + j + 1) * C]

                def tok(i):
                    return slice(off + i * C, off + (i + 1) * C)

                def move2(ps, lidx, relu=True):
                    # relu: t = max(psum*dsc, -B) (shifted; +B folded into
                    # the next layer's bias host-side).  no relu: full add.
                    dsc = float(2.0 ** (-scales[lidx])) \
                        if lidx in E3_STAGES else 1.0
                    xn = xp.tile([128, 2 * g * C], bf16, tag="x")
                    psv = ps[:].rearrange("p (a c) -> p a c", a=2 * g)
                    xnv = xn[:].rearrange("p (a c) -> p a c", a=2 * g)
                    b = bias2(lidx, s0, s1, neg=relu).broadcast_to(
                        [128, 2 * g, C])
                    nc.vector.scalar_tensor_tensor(
                        xnv, psv, dsc, b, MUL, MAX if relu else ADD)
                    return xn

                if stage == 0:  # w0 (fp16, 39-row slabs)
                    ps = psA.tile([128, 2 * g * C], f32, tag="mlp")
                    for i in range(g):
                        s = s0 + i
                        o = s * P_COLS
                        for j in (0, 1):
                            nc.tensor.matmul(
                                xsl(ps, i, j),
                                wp_sb[0:39, o + j * 128: o + j * 128 + 128],
                                embP[0:39, tok(i)], start=True, stop=True)
                    xs[wi_] = move2(ps, 0)
                elif stage in (1, 2, 3, 4, 6, 7):
                    ps = psA.tile([128, 2 * g * C], f32, tag="mlp")
                    xin = xs[wi_]
                    for i in range(g):
                        s = s0 + i
                        for j in (0, 1):
                            pj = xsl(ps, i, j)
                            nc.tensor.matmul(
                                pj, slab(stage, s, j * 128, j * 128 + 128),
                                xsl(xin, i, 0), start=True, stop=False)
                            nc.tensor.matmul(
                                pj, slab(stage, s, 256 + j * 128, 384 + j * 128),
                                xsl(xin, i, 1), start=False, stop=True)
                    xs[wi_] = move2(ps, stage)
                elif stage == 5:  # e3m4 mid + pre-scaled fp16 skip slab
                    ps = psA.tile([128, 2 * g * C], f32, tag="mlp")
                    xin = xs[wi_]
                    for i in range(g):
                        s = s0 + i
                        o = s * P_COLS
                        for j in (0, 1):
                            pj = xsl(ps, i, j)
                            nc.tensor.matmul(pj, slab(5, s, j * 128, j * 128 + 128),
                                             xsl(xin, i, 0),
                                             start=True, stop=False)
                            nc.tensor.matmul(pj, slab(5, s, 256 + j * 128, 384 + j * 128),
                                             xsl(xin, i, 1),
                                             start=False, stop=False)
                            nc.tensor.matmul(
                                pj,
                                wp_sb[0:39, o + 256 + j * 128: o + 384 + j * 128],
                                embP[0:39, tok(i)], start=False, stop=True)
                    xs[wi_] = move2(ps, 5)
                elif stage == 8:  # wi -> inter (e3m4; bias, no relu)
                    ps = psA.tile([128, 2 * g * C], f32, tag="mlp")
                    xin = xs[wi_]
                    for i in range(g):
                        s = s0 + i
                        for j in (0, 1):
                            pj = xsl(ps, i, j)
                            nc.tensor.matmul(pj, slab(8, s, j * 128, j * 128 + 128),
                                             xsl(xin, i, 0),
                                             start=True, stop=False)
                            nc.tensor.matmul(pj, slab(8, s, 256 + j * 128, 384 + j * 128),
                                             xsl(xin, i, 1),
                                             start=False, stop=True)
                    its[wi_] = move2(ps, 8, relu=False)
                elif stage == 9:  # wa -> alpha (fp16, tail chunk)
                    pa = psB.tile([3, g * C], f32, tag="head")
                    xin = xs[wi_]
                    for i in range(g):
                        s = s0 + i
                        o = s * TL_COLS
                        nc.tensor.matmul(pa[0:1, i * C:(i + 1) * C],
                                         wt_sb[:, o: o + 1],
                                         xsl(xin, i, 0),
                                         start=True, stop=False)
                        nc.tensor.matmul(pa[0:1, i * C:(i + 1) * C],
                                         wt_sb[:, o + 1: o + 2],
                                         xsl(xin, i, 1),
                                         start=False, stop=True)
                    av = alpha_sb[0:1, off: off + g * C].rearrange(
                        "p (g c) -> p g c", g=g)
                    pav = pa[0:1, :].rearrange("p (g c) -> p g c", g=g)
                    nc.vector.tensor_tensor(
                        av, pav,
                        bt_sb[0:1, 18 * nslot + s0: 18 * nslot + s1]
                        .broadcast_to([1, g, C]), ADD)
                elif stage == 10:  # wc0 (e3m4 inter + pre-scaled fp16 rays)
                    pc = psA.tile([128, g * C], f32, tag="mlp")
                    it = its[wi_]
                    for i in range(g):
                        s = s0 + i
                        op = s * P_COLS
                        pj = pc[:, i * C:(i + 1) * C]
                        nc.tensor.matmul(pj, slab(10, s, 0, 128),
                                         xsl(it, i, 0),
                                         start=True, stop=False)
                        nc.tensor.matmul(pj, slab(10, s, 128, 256),
                                         xsl(it, i, 1),
                                         start=False, stop=False)
                        nc.tensor.matmul(pj, wp_sb[0:27, op + 512: op + 640],
                                         embD[0:27, tok(i)],
                                         start=False, stop=True)
                    dsc = float(2.0 ** (-scales[10]))
                    ct = xp.tile([128, g * C], bf16, tag="ct")
                    pcv = pc[:].rearrange("p (g c) -> p g c", g=g)
                    ctv = ct[:].rearrange("p (g c) -> p g c", g=g)
                    bneg = bt_sb[:, NEG + 19 * nslot + s0: NEG + 19 * nslot + s1] \
                        .broadcast_to([128, g, C])
                    nc.vector.scalar_tensor_tensor(ctv, pcv, dsc, bneg, MUL, MAX)
                    cts[wi_] = ct
                elif stage == 11:  # wc1 -> sigmoid color (fp16, tail chunk)
                    pcol = psB.tile([3, g * C], f32, tag="head")
                    ct = cts[wi_]
                    for i in range(g):
                        s = s0 + i
                        o = s * TL_COLS
                        nc.tensor.matmul(pcol[:, i * C:(i + 1) * C],
                                         wt_sb[:, o + 2: o + 5],
                                         ct[:, i * C:(i + 1) * C],
                                         start=True, stop=True)
                    ctmp = xp.tile([3, g * C], f32, tag="ctmp")
                    pv = pcol[:].rearrange("p (g c) -> p g c", g=g)
                    cv = ctmp[:].rearrange("p (g c) -> p g c", g=g)
                    nc.vector.tensor_tensor(
                        cv, pv,
                        bt_sb[0:3, 20 * nslot + s0: 20 * nslot + s1]
                        .broadcast_to([3, g, C]), ADD)
                    nc.scalar.activation(color_sb[0:3, off: off + g * C],
                                         ctmp[:], Sigmoid)

            for stage in range(12):
                for wi_ in range(nw):
                    emit_stage(wi_, stage)

            nc.sync.dma_start(al_d.ap()[:], alpha_sb[:])
            nc.sync.dma_start(co_d.ap()[:], color_sb[:])

    nc.compile()
    return nc


_prog_cache = {}
_last_results = None


def _get_program(waves, nslot, scales):
    key = (tuple(waves), nslot, tuple(sorted(scales.items())))
    if key not in _prog_cache:
        _prog_cache[key] = _build_program(waves, nslot, scales)
    return _prog_cache[key]


# ---------------------------------------------------------------------------
# Host wrapper
# ---------------------------------------------------------------------------

def kernel(**inputs):
    global _last_results
    inputs = {k: np.asarray(v) for k, v in inputs.items()}
    idx = inputs["index"].astype(np.int64)
    B = idx.shape[0]
    points = inputs["points"].astype(np.float32)
    dirs = inputs["directions"].astype(np.float32)

    # --- routing: split each expert's tokens into <=CAP_MAX chunks ("virtual
    # experts"), distribute round-robin (sorted by size) over 8 cores ---
    tok = [np.nonzero(idx == e)[0] for e in range(E)]
    virt = []  # (expert, token_ids)
    for e in range(E):
        t = tok[e]
        if len(t) == 0:
            continue
        for lo in range(0, len(t), CAP_MAX):
            virt.append((e, t[lo: lo + CAP_MAX]))
    if not virt:
        virt = [(0, np.zeros((0,), np.int64))]
    virt.sort(key=lambda v: -len(v[1]))
    nslot = max(1, int(np.ceil(len(virt) / N_CORES)))

    core_slots = [[] for _ in range(N_CORES)]
    for i, v in enumerate(virt):
        core_slots[i % N_CORES].append(v)

    # per-slot-index capacity = max token count across cores (sorted deal
    # means slot i's max is virt[8i])
    slot_cap = [len(virt[N_CORES * i][1]) if N_CORES * i < len(virt) else 1
                for i in range(nslot)]
    waves = _make_waves(nslot, slot_cap)
    nall = sum((s1 - s0) * cw for s0, s1, cw, _ in waves)
    # slot -> (column offset, capacity)
    slot_pos = {}
    for s0, s1, cw, off in waves:
        for i in range(s1 - s0):
            slot_pos[s0 + i] = (off + i * cw, cw)

    # per-layer global pow2 scales for the e3m4 stages (w5/wc0 scale over
    # the e3m4 rows only; their fp16 skip/rays slabs reuse the same k)
    scales = {}
    for st, name in ((1, "w1"), (2, "w2"), (3, "w3"), (4, "w4"),
                     (6, "w6"), (7, "w7"), (8, "wi")):
        gmax = float(np.abs(inputs[name]).max())
        scales[st] = float(np.floor(np.log2(TARGET / gmax)))
    scales[5] = float(np.floor(np.log2(
        TARGET / float(np.abs(inputs["w5"][:, 0:256]).max()))))
    scales[10] = float(np.floor(np.log2(
        TARGET / float(np.abs(inputs["wc0"][:, 0:256]).max()))))

    nc = _get_program(waves, nslot, scales)

    # device-exact scaled weights (WS, written into the chunk arrays) and
    # dequantized-effective weights (for bias folding)
    e3np = ml_dtypes.float8_e3m4
    WS, WQ = {}, {}
    for st, name in ((1, "w1"), (2, "w2"), (3, "w3"), (4, "w4"),
                     (6, "w6"), (7, "w7"), (8, "wi")):
        sc = 2.0 ** scales[st]
        q = (inputs[name].astype(np.float32) * sc).astype(e3np) \
            .astype(np.float32)
        WS[name] = q
        WQ[name] = q / sc
    sc5 = 2.0 ** scales[5]
    q5 = (inputs["w5"][:, 0:256].astype(np.float32) * sc5).astype(e3np) \
        .astype(np.float32)
    WS["w5"] = q5
    WQ["w5"] = q5 / sc5
    sc10 = 2.0 ** scales[10]
    q10 = (inputs["wc0"][:, 0:256].astype(np.float32) * sc10).astype(e3np) \
        .astype(np.float32)
    WS["wc0"] = q10

    # folded biases: B_l = b_l + Wq_l^T B_{l-1} (exact quantized weights);
    # wa/wc1 consume shifted inputs too (fp16-rounded on device)
    FB = {}
    Bprev = inputs["b0"].astype(np.float64)
    FB["b0"] = Bprev.astype(np.float32)
    for l in range(1, 8):
        Wd = WQ[f"w{l}"].astype(np.float64) if l != 5 else \
            WQ["w5"].astype(np.float64)
        Bprev = inputs[f"b{l}"].astype(np.float64) + \
            np.einsum("eio,ei->eo", Wd, Bprev)
        FB[f"b{l}"] = Bprev.astype(np.float32)
    FB["bi"] = (inputs["bi"].astype(np.float64) + np.einsum(
        "eio,ei->eo", WQ["wi"].astype(np.float64), Bprev)).astype(np.float32)
    wa16 = inputs["wa"].astype(np.float16).astype(np.float64)
    FB["ba"] = (inputs["ba"].astype(np.float64) + np.einsum(
        "eio,ei->eo", wa16, Bprev)).astype(np.float32)
    wc116 = inputs["wc1"].astype(np.float16).astype(np.float64)
    FB["bc1"] = (inputs["bc1"].astype(np.float64) + np.einsum(
        "eio,ei->eo", wc116, inputs["bc0"].astype(np.float64))) \
        .astype(np.float32)

    # frequency expansion blocks with the pi/2 phase row (row 3):
    # pts: cols 0:18 sin, 18:36 cos; dirs: 0:12 sin, 12:24 cos
    fx2 = np.zeros((4, 36), np.float32)
    for c in range(3):
        for k in range(NX):
            fx2[c, c * NX + k] = float(2 ** k)
            fx2[c, 18 + c * NX + k] = float(2 ** k)
    fx2[3, 18:36] = PI / 2
    fd2 = np.zeros((4, 24), np.float32)
    for c in range(3):
        for k in range(ND):
            fd2[c, c * ND + k] = float(2 ** k)
            fd2[c, 12 + c * ND + k] = float(2 ** k)
    fd2[3, 12:24] = PI / 2

    e3 = ml_dtypes.float8_e3m4
    in_maps = []
    for cid in range(N_CORES):
        chunks = {"P": np.zeros((39, nslot * P_COLS), np.float32),
                  "T": np.zeros((128, nslot * TL_COLS), np.float32),
                  "E3": np.zeros((128, nslot * E3_COLS), np.float32)}
        bt = np.zeros((128, NB * nslot), np.float32)
        pd = np.zeros((4, 2 * nall + 60), np.float32)
        pd[3, 0: 2 * nall] = 1.0
        pd[:, 2 * nall: 2 * nall + 36] = fx2
        pd[:, 2 * nall + 36: 2 * nall + 60] = fd2
        xyzb = np.zeros((6, nall), np.float32)
        for s, (e, t) in enumerate(core_slots[cid]):
            _pack_expert(chunks, bt, s, nslot, inputs, e, scales, WS, FB)
            n = len(t)
            if n:
                o, _ = slot_pos[s]
                pd[0:3, o: o + n] = points[t].T
                pd[0:3, nall + o: nall + o + n] = dirs[t].T
                xyzb[0:3, o: o + n] = points[t].T
                xyzb[3:6, o: o + n] = dirs[t].T
        im = {"pd": pd,
              "bt": np.concatenate([bt, -bt], axis=1),
              "xyzb": xyzb.astype(ml_dtypes.bfloat16),
              "wP": chunks["P"].astype(np.float16),
              "wT": chunks["T"].astype(np.float16),
              "wE3": chunks["E3"].astype(e3)}
        in_maps.append(im)

    res = run_bass_kernel_spmd(nc, in_maps, core_ids=list(range(N_CORES)))
    _last_results = res

    out = np.zeros((B, 4), np.float32)
    for cid in range(N_CORES):
        al = res.results[cid]["alpha_out"]
        co = res.results[cid]["color_out"]
        for s, (e, t) in enumerate(core_slots[cid]):
            n = len(t)
            if n:
                o, _ = slot_pos[s]
                out[t, 0] = al[0, o: o + n]
                out[t, 1:4] = co[:, o: o + n].T
    return out


# revision 16
# speedup vs baseline: 1.1919x; 1.0244x over previous
"""NerfExperts MoE kernel for Trainium2, expert-parallel over 8 NeuronCores.

Each of the 1024 points is routed to one of 100 experts (~2.3MB fp32 of
weights each -> memory bound).  Experts are sharded across the 8 cores
(~13 slots/core); tokens are dispatched host-side; each expert's weights
stream from HBM exactly once in mixed precision:

  * w1-w7(mid), wi, wc0(inter) -> fp8 e3m4, scaled by a per-layer power
    of two (max |W| ~ 7, e3m4's normal band); the 2^-k descale rides the
    PSUM->SBUF move.  The fp16 skip/rays slabs that accumulate into the
    same PSUM groups are pre-scaled by the same 2^k (exact in fp16).
  * w0, w5skip, wc0rays, wa, wc1 -> fp16.  biases fp32.  Activations
    stay bf16 (mixed-dtype matmul is legal; only fp32 must match).

Weight DMA is ~8.0MB per core.  The two HWDGE rings (sync + scalar)
carry one ~0.85MB e3m4 slice per layer, alternating rings so the
cumulative arrival curve tracks stage consumption; the tiny inputs ride
SWDGE (gpsimd) so neither HWDGE ring's 6-deep kick window stalls on
them.  (SWDGE throttles SDMA packet rates while active, but it only
carries ~1MB up front.)

The per-stage dependency chain is PE -> one DVE op -> PE: the layer
output is stored SHIFTED as t = max(psum * 2^-k, -B) (= relu(z) - B),
and the missing "+B" is folded into the NEXT layer's bias on the host
(B_{l+1} = b_{l+1} + Wq_{l+1}^T B_l with the exact quantized weights).
No ACT relu, no second DVE op.  Each stage-wave keeps TWO PSUM tiles
(output halves j=0/j=1) so consecutive matmul accumulation groups
alternate PSUM banks, keeping the PE pipeline deep; the bias table and
x tiles are j-major to match.

Harmonic embedding: one fused chain per source -- the frequency matmul
carries a constant pi/2 "phase row", so sin rows 0:18 and cos rows
18:36 come from one range-reduction + ACT Sin pass (the ACT ring only
has 4 weight kicks ahead of it).

Embedding tile rows: pts: sin 0:18, cos 18:36, xyz 36:39;
dirs: sin 0:12, cos 12:24, xyz 24:27.
"""

import numpy as np
import ml_dtypes

import concourse.bass as bass
import concourse.bacc as bacc
import concourse.mybir as mybir
import concourse.tile as tile
from concourse.bass_utils import run_bass_kernel_spmd

PI = float(np.pi)
N_CORES = 8
E = 100
NX, ND = 6, 4
CAP_MAX = 128  # max tokens per expert slot
TARGET = 10.0  # e3m4 scale target for max|W| (normal band, <15.5)

# stages with e3m4 weights (descale 2^-k in the PSUM->SBUF move)
E3_STAGES = (1, 2, 3, 4, 5, 6, 7, 8)
E3_COLS = 8 * 512 + 256          # per-slot columns in the e3m4 mega tensor

# fp32 bias tensor [128, 2*NB*nslot]: first NB*nslot cols as below, then
# the same columns negated (for the DVE relu max-trick).
#   mlp stage lidx in 0..8: stage block at lidx*2*nslot, inside it the
#   columns are wave-major then J-MAJOR: col = 2*woff_w + j*g_w + i
#   ba: 18*nslot + s ; bc0: 19*nslot + s ; bc1: 20*nslot + s
NB = 21

P_COLS = 640   # per slot: w0 256 | w5skip 256 | rays 128  (39 rows, fp16)
TL_COLS = 5    # per slot: wa 2 | wc1 3                    (128 rows, fp16)


def _e3_off(st, nslot):
    # column offset of stage block in the e3m4 mega tensor
    # st in 1..7 -> mid layers; 8 -> wi; 10 -> wc0 inter
    if st == 10:
        return 8 * 512 * nslot
    return (st - 1) * 512 * nslot


def _bias_col(lidx, waves, nslot, s, j):
    """j-major bias column for (stage lidx, slot s, half j)."""
    for s0, s1, cw, off in waves:
        if s0 <= s < s1:
            g = s1 - s0
            return lidx * 2 * nslot + 2 * s0 + j * g + (s - s0)
    raise ValueError(s)


def _pack_expert(chunks, bt, s, nslot, inputs, e, scales, WS, FB, waves):
    """Fill slot s of the per-chunk host arrays (fp32; cast later).
    WS[name] holds the device-exact scaled weights (quantize-idempotent
    for e3m4 names); FB holds the folded biases."""

    def set_b2(lidx, b):
        bt[:, _bias_col(lidx, waves, nslot, s, 0)] = b[0:128]
        bt[:, _bias_col(lidx, waves, nslot, s, 1)] = b[128:256]

    # --- P chunk (39 rows): w0 | w5 skip part | wc0 rays part.  skip and
    # rays are pre-scaled by their stage's 2^k so the shared PSUM
    # accumulation with the e3m4 slabs stays uniform. ---
    p = chunks["P"]
    o = s * P_COLS
    p[0:39, o: o + 256] = inputs["w0"][e]                 # [39, 256]
    p[0:39, o + 256: o + 512] = inputs["w5"][e][256:295] * 2.0 ** scales[5]
    p[0:27, o + 512: o + 640] = inputs["wc0"][e][256:283] * 2.0 ** scales[10]
    set_b2(0, FB["b0"][e])
    # --- e3m4 mega tensor: 512-col blocks (k-split halves of [256,256]) ---
    w8 = chunks["E3"]
    for st, name in ((1, "w1"), (2, "w2"), (3, "w3"), (4, "w4"),
                     (5, "w5"), (6, "w6"), (7, "w7"), (8, "wi")):
        w = WS[name][e]
        o = _e3_off(st, nslot) + s * 512
        for k in (0, 1):
            w8[:, o + k * 256: o + (k + 1) * 256] = w[128 * k: 128 * (k + 1)]
        set_b2(st, FB["b5" if name == "w5" else
                   ("bi" if name == "wi" else f"b{st}")][e])
    o = _e3_off(10, nslot) + s * 256
    wc0s = WS["wc0"][e]
    w8[:, o: o + 128] = wc0s[0:128]
    w8[:, o + 128: o + 256] = wc0s[128:256]
    bt[:, 19 * nslot + s] = inputs["bc0"][e]
    # --- tiny fp16 tail: wa | wc1 ---
    t = chunks["T"]
    o = s * TL_COLS
    wa = inputs["wa"][e][:, 0]
    t[:, o] = wa[0:128]
    t[:, o + 1] = wa[128:256]
    bt[0, 18 * nslot + s] = FB["ba"][e][0]
    t[:, o + 2: o + 5] = inputs["wc1"][e]
    bt[0:3, 20 * nslot + s] = FB["bc1"][e]


def _make_waves(nslot, slot_cap):
    """Wave list [(s0, s1, Cw, off)] with per-wave capacity."""
    Cmax = max(4, int(np.ceil(max(slot_cap) / 4) * 4))
    gmax = max(1, min(512 // (2 * Cmax), 6))
    nw = int(np.ceil(nslot / gmax))
    base = nslot // nw
    rem = nslot - base * nw
    sizes = [base + (1 if i < rem else 0) for i in range(nw)]
    waves, s0, off = [], 0, 0
    for g in sizes:
        cw = max(4, int(np.ceil(max(slot_cap[s0:s0 + g]) / 4) * 4))
        waves.append((s0, s0 + g, cw, off))
        off += g * cw
        s0 += g
    return waves


# ---------------------------------------------------------------------------
# Device program
# ---------------------------------------------------------------------------

def _build_program(waves, nslot, scales):
    """SPMD Bass program: nslot expert slots, per-wave token capacity."""
    waves = list(waves)
    nall = sum((s1 - s0) * cw for s0, s1, cw, _ in waves)
    nw = len(waves)
    f32 = mybir.dt.float32
    f16 = mybir.dt.float16
    bf16 = mybir.dt.bfloat16
    f8e3 = mybir.dt.float8e3
    Sin = mybir.ActivationFunctionType.Sin
    Sigmoid = mybir.ActivationFunctionType.Sigmoid
    ADD = mybir.AluOpType.add
    SUB = mybir.AluOpType.subtract
    MUL = mybir.AluOpType.mult
    MAX = mybir.AluOpType.max
    MIN = mybir.AluOpType.min
    # range-reduction constants (Cody-Waite, fp32 magic rounding)
    INV2PI = float(np.float32(1.0 / (2 * PI)))
    MAGIC = 12582912.0            # 1.5 * 2**23: forces round-to-int in fp32
    C1 = 6.28125                  # 2*pi high part, exact in fp32
    C2 = float(np.float32(2 * PI - 6.28125))
    CLAMP = 3.1415925             # just under pi (ACT Sin domain is [-pi, pi])

    nc = bacc.Bacc("TRN2", target_bir_lowering=False, debug=False)
    pd_d = nc.dram_tensor("pd", (4, 2 * nall + 60), f32, kind="ExternalInput")
    xyz_d = nc.dram_tensor("xyzb", (6, nall), bf16, kind="ExternalInput")
    bt_d = nc.dram_tensor("bt", (128, 2 * NB * nslot), f32,
                          kind="ExternalInput")
    wp_d = nc.dram_tensor("wP", (39, nslot * P_COLS), f16, kind="ExternalInput")
    we3_d = nc.dram_tensor("wE3", (128, nslot * E3_COLS), f8e3,
                           kind="ExternalInput")
    wt_d = nc.dram_tensor("wT", (128, nslot * TL_COLS), f16,
                          kind="ExternalInput")
    al_d = nc.dram_tensor("alpha_out", (1, nall), f32, kind="ExternalOutput")
    co_d = nc.dram_tensor("color_out", (3, nall), f32, kind="ExternalOutput")

    with tile.TileContext(nc) as tc:
        with (
            tc.tile_pool(name="cp", bufs=1) as cp,
            tc.tile_pool(name="xp", bufs=4 * nw + 2) as xp,
            tc.tile_pool(name="ps8", bufs=8, space=bass.MemorySpace.PSUM) as ps8,
        ):
            embP = cp.tile([39, nall], bf16)  # sin 0:18, cos 18:36, xyz 36:39
            embD = cp.tile([27, nall], bf16)  # sin 0:12, cos 12:24, xyz 24:27
            pd_sb = cp.tile([4, 2 * nall + 60], f32)
            bt_sb = cp.tile([128, 2 * NB * nslot], f32)
            wp_sb = cp.tile([39, nslot * P_COLS], f16, name="wP", tag="wP")
            we3_sb = cp.tile([128, nslot * E3_COLS], f8e3, name="wE3",
                             tag="wE3")
            wt_sb = cp.tile([128, nslot * TL_COLS], f16, name="wT", tag="wT")

            # ---- DMA kicks, all before any compute.  Weights only on the
            # two HWDGE rings (<=6 kicks each, inside the ring window);
            # small inputs on SWDGE so they never block a weight kick. ----
            nc.sync.dma_start(pd_sb[:], pd_d.ap()[:])
            nc.gpsimd.dma_start(embP[36:39, :], xyz_d.ap()[0:3, :])
            nc.gpsimd.dma_start(embD[24:27, :], xyz_d.ap()[3:6, :])
            nc.gpsimd.dma_start(bt_sb[:], bt_d.ap()[:])
            nc.gpsimd.dma_start(wt_sb[:], wt_d.ap()[:])
            nc.gpsimd.dma_start(wp_sb[:], wp_d.ap()[:])

            def e3_slice(eng, st):
                lo = _e3_off(st, nslot)
                hi = _e3_off(st + 1, nslot) if st < 8 else (
                    _e3_off(10, nslot) if st == 8 else nslot * E3_COLS)
                eng.dma_start(we3_sb[:, lo:hi], we3_d.ap()[:, lo:hi])

            for st in (1, 3, 5, 7, 10):
                e3_slice(nc.sync, st)
            for st in (2, 4, 6, 8):
                e3_slice(nc.scalar, st)

            alpha_sb = cp.tile([1, nall], f32)
            color_sb = cp.tile([3, nall], f32)

            # ---- harmonic embedding: ONE fused sin+cos chain per source.
            # ep rows 0:half are freq*x, rows half:2*half are freq*x + pi/2
            # (phase row trick), so Sin() yields sin|cos in one shot. ----
            def reduce_sin(tsrc, rows, ncol):
                t1 = xp.tile([rows, ncol], f32, tag="vred")
                nc.vector.tensor_scalar(t1[:], tsrc, INV2PI, MAGIC, MUL, ADD)
                r = xp.tile([rows, ncol], f32, tag="vred")
                nc.vector.tensor_scalar(r[:], t1[:], MAGIC, None, SUB)
                rd = xp.tile([rows, ncol], f32, tag="vred")
                nc.vector.scalar_tensor_tensor(rd[:], r[:], -C1, tsrc, MUL, ADD)
                rd2 = xp.tile([rows, ncol], f32, tag="vred")
                nc.vector.scalar_tensor_tensor(rd2[:], r[:], -C2, rd[:], MUL, ADD)
                v = xp.tile([rows, ncol], f32, tag="vred")
                nc.vector.tensor_scalar(v[:], rd2[:], CLAMP, -CLAMP, MIN, MAX)
                return v

            for lo in range(0, nall, 512):
                hi = min(nall, lo + 512)
                w_ = hi - lo
                for (rows, fcol, src_lo, dst) in (
                    (36, 2 * nall, 0, embP),
                    (24, 2 * nall + 36, nall, embD),
                ):
                    ep = ps8.tile([rows, w_], f32, tag="mlp")
                    nc.tensor.matmul(ep[:], pd_sb[0:4, fcol: fcol + rows],
                                     pd_sb[0:4, src_lo + lo: src_lo + hi],
                                     start=True, stop=True)
                    vs = reduce_sin(ep[:], rows, w_)
                    nc.scalar.activation(dst[0:rows, lo:hi], vs[:], Sin)

            # ---- wave-lockstep MLP.  One DVE op per (wave, j-half):
            # t = max(psum*dsc, -B) with the +B folded into the next
            # layer's bias host-side.  Two PSUM tiles per stage-wave
            # (j=0/j=1) so consecutive accumulation groups alternate
            # PSUM banks and the PE pipeline stays deep. ----
            NEG = NB * nslot  # column offset of the negated bias copy

            xs = [None] * nw   # pairs (xnA, xnB)
            its = [None] * nw
            cts = [None] * nw

            def slab(st, s, lo, hi):
                o = _e3_off(st, nslot) + s * (256 if st == 10 else 512)
                return we3_sb[0:128, o + lo: o + hi]

            def emit_stage(wi_, stage):
                s0, s1, C, off = waves[wi_]
                g = s1 - s0

                def tok(i):
                    return slice(off + i * C, off + (i + 1) * C)

                def bias1(base, neg=False, p=128):
                    b = (NEG if neg else 0) + base
                    return bt_sb[0:p, b + s0: b + s1]

                def biasj(lidx, j, neg=False):
                    b = (NEG if neg else 0) + lidx * 2 * nslot + 2 * s0 + j * g
                    return bt_sb[:, b: b + g]

                def move1(psj, lidx, j, relu=True):
                    # one DVE op per j-half: t = max(psum*dsc, -B) | +B
                    dsc = float(2.0 ** (-scales[lidx])) \
                        if lidx in E3_STAGES else 1.0
                    xn = xp.tile([128, g * C], bf16, tag="x")
                    psv = psj[:].rearrange("p (g c) -> p g c", g=g)
                    xnv = xn[:].rearrange("p (g c) -> p g c", g=g)
                    b = biasj(lidx, j, neg=relu).broadcast_to([128, g, C])
                    nc.vector.scalar_tensor_tensor(
                        xnv, psv, dsc, b, MUL, MAX if relu else ADD)
                    return xn

                if stage == 0:  # w0 (fp16, 39-row slabs)
                    psj = [ps8.tile([128, g * C], f32, tag="mlp",
                                    name=f"ps{stage}w{wi_}j{jj}")
                           for jj in (0, 1)]
                    for i in range(g):
                        s = s0 + i
                        o = s * P_COLS
                        for j in (0, 1):
                            nc.tensor.matmul(
                                psj[j][:, i * C:(i + 1) * C],
                                wp_sb[0:39, o + j * 128: o + j * 128 + 128],
                                embP[0:39, tok(i)], start=True, stop=True)
                    xs[wi_] = (move1(psj[0], 0, 0), move1(psj[1], 0, 1))
                elif stage in (1, 2, 3, 4, 6, 7, 8):
                    psj = [ps8.tile([128, g * C], f32, tag="mlp",
                                    name=f"ps{stage}w{wi_}j{jj}")
                           for jj in (0, 1)]
                    xin = xs[wi_]
                    for i in range(g):
                        s = s0 + i
                        for j in (0, 1):
                            pj = psj[j][:, i * C:(i + 1) * C]
                            nc.tensor.matmul(
                                pj, slab(stage, s, j * 128, j * 128 + 128),
                                xin[0][:, i * C:(i + 1) * C],
                                start=True, stop=False)
                            nc.tensor.matmul(
                                pj, slab(stage, s, 256 + j * 128, 384 + j * 128),
                                xin[1][:, i * C:(i + 1) * C],
                                start=False, stop=True)
                    relu = stage != 8
                    out = (move1(psj[0], stage, 0, relu),
                           move1(psj[1], stage, 1, relu))
                    if stage == 8:
                        its[wi_] = out
                    else:
                        xs[wi_] = out
                elif stage == 5:  # e3m4 mid + pre-scaled fp16 skip slab
                    psj = [ps8.tile([128, g * C], f32, tag="mlp",
                                    name=f"ps{stage}w{wi_}j{jj}")
                           for jj in (0, 1)]
                    xin = xs[wi_]
                    for i in range(g):
                        s = s0 + i
                        o = s * P_COLS
                        for j in (0, 1):
                            pj = psj[j][:, i * C:(i + 1) * C]
                            nc.tensor.matmul(pj, slab(5, s, j * 128, j * 128 + 128),
                                             xin[0][:, i * C:(i + 1) * C],
                                             start=True, stop=False)
                            nc.tensor.matmul(pj, slab(5, s, 256 + j * 128, 384 + j * 128),
                                             xin[1][:, i * C:(i + 1) * C],
                                             start=False, stop=False)
                            nc.tensor.matmul(
                                pj,
                                wp_sb[0:39, o + 256 + j * 128: o + 384 + j * 128],
                                embP[0:39, tok(i)], start=False, stop=True)
                    xs[wi_] = (move1(psj[0], 5, 0), move1(psj[1], 5, 1))
                elif stage == 9:  # wa -> alpha (fp16, tail chunk)
                    pa = ps8.tile([3, g * C], f32, tag="mlp")
                    xin = xs[wi_]
                    for i in range(g):
                        s = s0 + i
                        o = s * TL_COLS
                        nc.tensor.matmul(pa[0:1, i * C:(i + 1) * C],
                                         wt_sb[:, o: o + 1],
                                         xin[0][:, i * C:(i + 1) * C],
                                         start=True, stop=False)
                        nc.tensor.matmul(pa[0:1, i * C:(i + 1) * C],
                                         wt_sb[:, o + 1: o + 2],
                                         xin[1][:, i * C:(i + 1) * C],
                                         start=False, stop=True)
                    av = alpha_sb[0:1, off: off + g * C].rearrange(
                        "p (g c) -> p g c", g=g)
                    pav = pa[0:1, :].rearrange("p (g c) -> p g c", g=g)
                    nc.vector.tensor_tensor(
                        av, pav, bias1(18 * nslot, p=1).broadcast_to([1, g, C]),
                        ADD)
                elif stage == 10:  # wc0 (e3m4 inter + pre-scaled fp16 rays)
                    pc = ps8.tile([128, g * C], f32, tag="mlp")
                    it = its[wi_]
                    for i in range(g):
                        s = s0 + i
                        op = s * P_COLS
                        pj = pc[:, i * C:(i + 1) * C]
                        nc.tensor.matmul(pj, slab(10, s, 0, 128),
                                         it[0][:, i * C:(i + 1) * C],
                                         start=True, stop=False)
                        nc.tensor.matmul(pj, slab(10, s, 128, 256),
                                         it[1][:, i * C:(i + 1) * C],
                                         start=False, stop=False)
                        nc.tensor.matmul(pj, wp_sb[0:27, op + 512: op + 640],
                                         embD[0:27, tok(i)],
                                         start=False, stop=True)
                    dsc = float(2.0 ** (-scales[10]))
                    ct = xp.tile([128, g * C], bf16, tag="ct")
                    pcv = pc[:].rearrange("p (g c) -> p g c", g=g)
                    ctv = ct[:].rearrange("p (g c) -> p g c", g=g)
                    nc.vector.scalar_tensor_tensor(
                        ctv, pcv, dsc,
                        bias1(19 * nslot, neg=True).broadcast_to([128, g, C]),
                        MUL, MAX)
                    cts[wi_] = ct
                elif stage == 11:  # wc1 -> sigmoid color (fp16, tail chunk)
                    pcol = ps8.tile([3, g * C], f32, tag="mlp")
                    ct = cts[wi_]
                    for i in range(g):
                        s = s0 + i
                        o = s * TL_COLS
                        nc.tensor.matmul(pcol[:, i * C:(i + 1) * C],
                                         wt_sb[:, o + 2: o + 5],
                                         ct[:, i * C:(i + 1) * C],
                                         start=True, stop=True)
                    ctmp = xp.tile([3, g * C], f32, tag="ctmp")
                    pv = pcol[:].rearrange("p (g c) -> p g c", g=g)
                    cv = ctmp[:].rearrange("p (g c) -> p g c", g=g)
                    nc.vector.tensor_tensor(
                        cv, pv, bias1(20 * nslot, p=3).broadcast_to([3, g, C]),
                        ADD)
                    nc.scalar.activation(color_sb[0:3, off: off + g * C],
                                         ctmp[:], Sigmoid)

            for stage in range(12):
                for wi_ in range(nw):
                    emit_stage(wi_, stage)

            nc.sync.dma_start(al_d.ap()[:], alpha_sb[:])
            nc.sync.dma_start(co_d.ap()[:], color_sb[:])

    nc.compile()
    return nc


_prog_cache = {}
_last_results = None


def _get_program(waves, nslot, scales):
    key = (tuple(waves), nslot, tuple(sorted(scales.items())))
    if key not in _prog_cache:
        _prog_cache[key] = _build_program(waves, nslot, scales)
    return _prog_cache[key]


# ---------------------------------------------------------------------------
# Host wrapper
# ---------------------------------------------------------------------------

def kernel(**inputs):
    global _last_results
    inputs = {k: np.asarray(v) for k, v in inputs.items()}
    idx = inputs["index"].astype(np.int64)
    B = idx.shape[0]
    points = inputs["points"].astype(np.float32)
    dirs = inputs["directions"].astype(np.float32)

    # --- routing: split each expert's tokens into <=CAP_MAX chunks ("virtual
    # experts"), distribute round-robin (sorted by size) over 8 cores ---
    tok = [np.nonzero(idx == e)[0] for e in range(E)]
    virt = []  # (expert, token_ids)
    for e in range(E):
        t = tok[e]
        if len(t) == 0:
            continue
        for lo in range(0, len(t), CAP_MAX):
            virt.append((e, t[lo: lo + CAP_MAX]))
    if not virt:
        virt = [(0, np.zeros((0,), np.int64))]
    virt.sort(key=lambda v: -len(v[1]))
    nslot = max(1, int(np.ceil(len(virt) / N_CORES)))

    core_slots = [[] for _ in range(N_CORES)]
    for i, v in enumerate(virt):
        core_slots[i % N_CORES].append(v)

    # per-slot-index capacity = max token count across cores (sorted deal
    # means slot i's max is virt[8i])
    slot_cap = [len(virt[N_CORES * i][1]) if N_CORES * i < len(virt) else 1
                for i in range(nslot)]
    waves = _make_waves(nslot, slot_cap)
    nall = sum((s1 - s0) * cw for s0, s1, cw, _ in waves)
    # slot -> (column offset, capacity)
    slot_pos = {}
    for s0, s1, cw, off in waves:
        for i in range(s1 - s0):
            slot_pos[s0 + i] = (off + i * cw, cw)

    # per-layer global pow2 scales for the e3m4 stages (w5/wc0 scale over
    # the e3m4 rows only; their fp16 skip/rays slabs reuse the same k)
    scales = {}
    for st, name in ((1, "w1"), (2, "w2"), (3, "w3"), (4, "w4"),
                     (6, "w6"), (7, "w7"), (8, "wi")):
        gmax = float(np.abs(inputs[name]).max())
        scales[st] = float(np.floor(np.log2(TARGET / gmax)))
    scales[5] = float(np.floor(np.log2(
        TARGET / float(np.abs(inputs["w5"][:, 0:256]).max()))))
    scales[10] = float(np.floor(np.log2(
        TARGET / float(np.abs(inputs["wc0"][:, 0:256]).max()))))

    nc = _get_program(waves, nslot, scales)

    # device-exact scaled weights (WS, written into the chunk arrays) and
    # dequantized-effective weights (for bias folding)
    e3np = ml_dtypes.float8_e3m4
    WS, WQ = {}, {}
    for st, name in ((1, "w1"), (2, "w2"), (3, "w3"), (4, "w4"),
                     (6, "w6"), (7, "w7"), (8, "wi")):
        sc = 2.0 ** scales[st]
        q = (inputs[name].astype(np.float32) * sc).astype(e3np) \
            .astype(np.float32)
        WS[name] = q
        WQ[name] = q / sc
    sc5 = 2.0 ** scales[5]
    q5 = (inputs["w5"][:, 0:256].astype(np.float32) * sc5).astype(e3np) \
        .astype(np.float32)
    WS["w5"] = q5
    WQ["w5"] = q5 / sc5
    sc10 = 2.0 ** scales[10]
    q10 = (inputs["wc0"][:, 0:256].astype(np.float32) * sc10).astype(e3np) \
        .astype(np.float32)
    WS["wc0"] = q10

    # folded biases: B_l = b_l + Wq_l^T B_{l-1} (exact quantized weights);
    # wa/wc1 consume shifted inputs too (fp16-rounded on device)
    FB = {}
    Bprev = inputs["b0"].astype(np.float64)
    FB["b0"] = Bprev.astype(np.float32)
    for l in range(1, 8):
        Wd = WQ[f"w{l}"].astype(np.float64)
        Bprev = inputs[f"b{l}"].astype(np.float64) + \
            np.einsum("eio,ei->eo", Wd, Bprev)
        FB[f"b{l}"] = Bprev.astype(np.float32)
    FB["bi"] = (inputs["bi"].astype(np.float64) + np.einsum(
        "eio,ei->eo", WQ["wi"].astype(np.float64), Bprev)).astype(np.float32)
    wa16 = inputs["wa"].astype(np.float16).astype(np.float64)
    FB["ba"] = (inputs["ba"].astype(np.float64) + np.einsum(
        "eio,ei->eo", wa16, Bprev)).astype(np.float32)
    wc116 = inputs["wc1"].astype(np.float16).astype(np.float64)
    FB["bc1"] = (inputs["bc1"].astype(np.float64) + np.einsum(
        "eio,ei->eo", wc116, inputs["bc0"].astype(np.float64))) \
        .astype(np.float32)

    # frequency expansion blocks with the pi/2 phase row (row 3):
    # pts: cols 0:18 sin, 18:36 cos; dirs: 0:12 sin, 12:24 cos
    fx2 = np.zeros((4, 36), np.float32)
    for c in range(3):
        for k in range(NX):
            fx2[c, c * NX + k] = float(2 ** k)
            fx2[c, 18 + c * NX + k] = float(2 ** k)
    fx2[3, 18:36] = PI / 2
    fd2 = np.zeros((4, 24), np.float32)
    for c in range(3):
        for k in range(ND):
            fd2[c, c * ND + k] = float(2 ** k)
            fd2[c, 12 + c * ND + k] = float(2 ** k)
    fd2[3, 12:24] = PI / 2

    in_maps = []
    for cid in range(N_CORES):
        chunks = {"P": np.zeros((39, nslot * P_COLS), np.float32),
                  "T": np.zeros((128, nslot * TL_COLS), np.float32),
                  "E3": np.zeros((128, nslot * E3_COLS), np.float32)}
        bt = np.zeros((128, NB * nslot), np.float32)
        pd = np.zeros((4, 2 * nall + 60), np.float32)
        pd[3, 0: 2 * nall] = 1.0
        pd[:, 2 * nall: 2 * nall + 36] = fx2
        pd[:, 2 * nall + 36: 2 * nall + 60] = fd2
        xyzb = np.zeros((6, nall), np.float32)
        for s, (e, t) in enumerate(core_slots[cid]):
            _pack_expert(chunks, bt, s, nslot, inputs, e, scales, WS, FB,
                         waves)
            n = len(t)
            if n:
                o, _ = slot_pos[s]
                pd[0:3, o: o + n] = points[t].T
                pd[0:3, nall + o: nall + o + n] = dirs[t].T
                xyzb[0:3, o: o + n] = points[t].T
                xyzb[3:6, o: o + n] = dirs[t].T
        im = {"pd": pd,
              "bt": np.concatenate([bt, -bt], axis=1),
              "xyzb": xyzb.astype(ml_dtypes.bfloat16),
              "wP": chunks["P"].astype(np.float16),
              "wT": chunks["T"].astype(np.float16),
              "wE3": chunks["E3"].astype(e3np)}
        in_maps.append(im)

    res = run_bass_kernel_spmd(nc, in_maps, core_ids=list(range(N_CORES)))
    _last_results = res

    out = np.zeros((B, 4), np.float32)
    for cid in range(N_CORES):
        al = res.results[cid]["alpha_out"]
        co = res.results[cid]["color_out"]
        for s, (e, t) in enumerate(core_slots[cid]):
            n = len(t)
            if n:
                o, _ = slot_pos[s]
                out[t, 0] = al[0, o: o + n]
                out[t, 1:4] = co[:, o: o + n].T
    return out
